# revision 40
# baseline (speedup 1.0000x reference)
"""Trainium2 Bass kernel for nn_Block_79680233275670 (dense transformer block).

Reference, for x [16, 1024, 384]:
  x = x + proj(attn(LN1(x)))                               (4 heads, head_dim 96)
  x = x + fc2(hswish(dw3x3(hswish(fc1(LN2(x))))))          (IRB, 32x32 spatial)

Sharding: pure data-parallel over batch B=16 -> 8 cores x 2 batch items.
No collectives. Weights replicated (pre-transposed / LN-folded / fp8-quantized
host-side).

Per-core dataflow (T = 2048 tokens = 2 batches x 1024):
  - x token-major [128, 16, 384] f32 (4 chunked DMAs); residual stream f32
  - LN token-major (bn_stats) -> bf16, PE-transpose -> ACT fp8 cast
  - fp8e4 DoubleRow matmuls (2 k-tiles/pass) for QKV / PV / proj / fc1 / fc2;
    scores q^T k stay bf16
  - per (batch, head): scores St[m,n] = k^T q in a double-buffered PSUM;
    exp on ACT emits fp8 P m-tile pairs = the DoubleRow rhs for PV. An
    appended ones column in v (padded to 112 rows) makes PV also emit
    softmax denominators. O-normalization interleaved per head: DVE drain,
    PE transposes into one [128,8,98] PSUM tile, one recip, per-slice mults
  - proj/fc2 token-major; fp8 dequant scale folded into the
    scalar_tensor_tensor residual add (x += psum*s)
  - IRB: fc1 channel-major fp8; hardswish = ONE custom DVE op
    (min(relu(x*s+b+3),6)*(relu(..)-3), /6 folded into dw / fc2 weights)
    writing fp8 windows directly; depthwise 3x3 on PE over 19-row
    zero-padded windows (40-elem row pitch): 2 DoubleRow tap pairs
    ((-1,dx),(+1,dx) for dx=+-1, 80-elem k-stride satisfies the 16B rule)
    + 5 single taps
"""

import sys
import functools

for _p in ("/opt/trn_rl_repo",):
    if _p not in sys.path:
        sys.path.insert(0, _p)

import numpy as np
import ml_dtypes

import concourse.bass as bass
import concourse.mybir as mybir
import concourse.tile as tile
from concourse import bacc
from concourse.ap import AP
from concourse.masks import make_identity


B, N, C = 16, 1024, 384
HEADS, HD = 4, 96
VP = 112                   # v rows incl. ones col, padded for DR alignment
HID = 1536
NCORES = 8
BPC = B // NCORES          # batches per core
T = BPC * N                # tokens per core
NT = T // 128              # 16 token tiles per core
EPS = 1e-5

f32 = mybir.dt.float32
bf16 = mybir.dt.bfloat16
fp8 = mybir.dt.float8e4
AF = mybir.ActivationFunctionType
OP = mybir.AluOpType
DR = mybir.MatmulPerfMode.DoubleRow
nbf = ml_dtypes.bfloat16
nf8 = ml_dtypes.float8_e4m3

# ---- custom fused hardswish DVE op (registered at import time) ------------
# out = min(relu(in*C1 + C0), 6) * (relu(in*C1 + C0) - 3)
#     = hswish6(in*C1 + (C0-3)) where hswish6(x) = x*clip(x+3,0,6)
# C0 = bias+3 (per-partition AP), C1 = fp8 dequant scale, C2 = 6 (imm2),
# C3 = 3 delivered via in1 (spilled).
import concourse.dve_ops as dve_ops
from concourse.dve_spec import Spec, Src0, C0, C1, C2, C3, relu, minn, lower
from concourse.dve_ops import DveOp, DveOpSpec, _spill_c3_to_src1


def _register_hswish():
    name = "HSWISH6Q_ANT"
    if name in dve_ops._SUB_OPCODE_FOR_NAME:
        for op in dve_ops.OPS:
            if op.name == name:
                return op
    r = relu(Src0 * C1 + C0)
    spec = Spec(
        body=_spill_c3_to_src1(minn(r, C2) * (r - C3)),
        reference=lambda in0, in1, s0, s1, imm2:
        np.minimum(np.maximum(in0 * s1 + s0, 0), imm2)
        * (np.maximum(in0 * s1 + s0, 0) - in1),
    )
    op = DveOp(name, spec, subdim=False, uops_sha={})
    row = dve_ops._CUSTOM_DVE_ROW_BASE + len(dve_ops.OPS)
    assert row < 0x20
    for ver in ("v3", "v4"):
        probe = DveOpSpec(name=name, opcode=row, uops=lower(spec, ver=ver),
                          rd1_en=True)
        op.uops_sha[ver] = probe.sha(ver)
    dve_ops.OPS.append(op)
    dve_ops._SUB_OPCODE_FOR_NAME[name] = row
    dve_ops.CUSTOM_DVE_SPECS[name] = spec
    return op


HSWISH6Q = _register_hswish()

# ---- depthwise-window geometry -------------------------------------------
# 19 rows per window: row 0 and row 18 stay zero (vertical SAME padding),
# rows 1..17 hold 17 image rows (16 outputs + 1 halo). Rows padded to WP=40
# (32 data + 8 zero cols -> horizontal SAME padding), plus HOFF=3 leading
# zeros. All 9 taps share identical geometry:
#   acc[0:AUSE) += w_t * win[so : so+AUSE),  so = HOFF+(dy+1+yh)*WP+dx
# Taps (-1,dx) and (+1,dx) pair into one DoubleRow matmul (k-stride 2*WP=80,
# even offsets for dx=+-1 since HOFF=3).
WP = 40
HOFF = 3
HLEN = HOFF + 34 * WP + 1  # rows: 1 zero + 32 image rows + 1 zero (pad->/4)
ACCL = 32 * WP             # 1280 acc length (full batch image)
AUSE = ACCL - 2            # 1278 initialized prefix
SEGS3 = ((0, 512), (512, 1024), (1024, AUSE))
SEGS2A = ((0, 512), (512, 638))          # half-image variants (2-bank psum)
DW_PAIRS = ((0, 6), (2, 8))      # (dy=-1,dx) + (dy=+1,dx) for dx=-1,+1
DW_SINGLES = (1, 3, 4, 5, 7)


def tap_off(t):
    # window row w holds image row w-1; acc row y reads window row y+dy+1
    dy, dx = t // 3 - 1, t % 3 - 1
    return HOFF + (dy + 1) * WP + dx


def pair_ap(w, delta, lo, hi):
    """[128, 2, hi-lo] view: two copies of w[:, lo:hi] offset by delta elems."""
    v = w[:, lo:hi]
    return AP(v.tensor, v.offset, [list(v.ap[0]), [delta, 2], [1, hi - lo]])


def emit_kernel(nc, tc, d):
    from contextlib import ExitStack

    with ExitStack() as ctx:
        singles = ctx.enter_context(tc.tile_pool(name="singles", bufs=1))

        x_sb = singles.tile([128, NT, C], bf16)  # token-major; becomes x2 in place
        ident = singles.tile([128, 128], bf16)
        make_identity(nc, ident)
        ones8 = singles.tile([1, 128], fp8)
        nc.vector.memset(ones8, 1.0)
        eps_sb = singles.tile([128, 1], f32)
        nc.vector.memset(eps_sb, EPS)
        three = singles.tile([128, 1], f32)
        nc.vector.memset(three, 3.0)

        wqk_sb = singles.tile([128, 4, 2 * C], fp8)
        bqk_sb = singles.tile([96, 8], f32)
        wv_sb = singles.tile([128, 4, C], fp8)
        bv_sb = singles.tile([1, C], fp8)
        wp_sb = singles.tile([128, 4, C], fp8)
        bp_sb = singles.tile([1, C], fp8)
        wf1_sb = singles.tile([128, 4, HID], fp8)
        bf13_sb = singles.tile([128, 12], f32)
        wdgp_sb = singles.tile([128, 12, 4, 128], fp8)
        wdgs_sb = singles.tile([128, 12, 5, 128], fp8)
        bdw3_sb = singles.tile([128, 12], f32)
        wf2_sb = singles.tile([128, 12, C], fp8)
        bf2_sb = singles.tile([1, C], fp8)

        xn2_ch = singles.tile([128, 4, T], fp8)
        nc.vector.memset(xn2_ch[:, 3, :], 0.0)
        h1w_a = singles.tile([128, 12, HLEN], fp8)
        h1w_b = singles.tile([128, 12, HLEN], fp8)
        h1w_bufs = [h1w_a, h1w_b]

        # x[b, i*128+p, c] -> x_sb[p, b*8+i, c], 8 chunks on 2 queues so
        # LN1 starts early and the load runs on two DMA paths
        for ch in range(8):
            eng = nc.sync if ch % 2 == 0 else nc.scalar
            eng.dma_start(
                out=x_sb[:, ch * 2:(ch + 1) * 2, :],
                in_=d["x"].rearrange("b (i p) c -> p (b i) c", p=128)
                [:, ch * 2:(ch + 1) * 2, :],
            )
        for name, dst in (("wqk", wqk_sb), ("wv", wv_sb), ("wp", wp_sb)):
            nc.sync.dma_start(out=dst, in_=d[name].rearrange("k p m -> p k m"))
        for name, dst in (("bqk", bqk_sb), ("bv", bv_sb), ("bp", bp_sb),
                          ("bf13", bf13_sb), ("bdw3", bdw3_sb),
                          ("bf2", bf2_sb)):
            nc.sync.dma_start(out=dst, in_=d[name])
        nc.sync.dma_start(out=wf1_sb, in_=d["wf1"].rearrange("k p m -> p k m"))
        nc.sync.dma_start(out=wf2_sb, in_=d["wf2"].rearrange("k p m -> p k m"))
        nc.scalar.dma_start(
            out=wdgp_sb, in_=d["wdgp"].rearrange("m g c j -> c m g j"))
        nc.scalar.dma_start(
            out=wdgs_sb, in_=d["wdgs"].rearrange("m s c j -> c m s j"))

        SQ = d["scales"]  # dict of python floats
        HAS_BIAS = d["has_bias"]

        def ln_stats(tts, ln_pool):
            # one Sqrt + one reciprocal for the whole tile group
            G = len(tts)
            mvs = ln_pool.tile([128, 8, 2], f32, tag="ln_mvs")
            for i, tt in enumerate(tts):
                stats = ln_pool.tile([128, 6], f32, tag="ln_stats")
                nc.vector.bn_stats(stats, x_sb[:, tt, :])
                nc.vector.bn_aggr(mvs[:, i, :], stats)
            stds = ln_pool.tile([128, 8], f32, tag="ln_stds")
            nc.scalar.activation(stds[:, 0:G], mvs[:, 0:G, 1], AF.Sqrt,
                                 bias=eps_sb)
            rstds = ln_pool.tile([128, 8], f32, tag="ln_rstds")
            nc.vector.reciprocal(rstds[:, 0:G], stds[:, 0:G])
            return mvs, rstds

        def ln_tile(xn_ch, tt, i, mvs, rstds, ln_pool, ps_pool,
                    xn_drain=None, affine="dve"):
            xn = ln_pool.tile([128, C], bf16, tag="ln_xn")
            if affine == "act":
                negmr = ln_pool.tile([128, 1], f32, tag="ln_negmr")
                nc.vector.tensor_scalar(
                    negmr, mvs[:, i, 0:1], -1.0, rstds[:, i:i + 1],
                    OP.mult, OP.mult
                )
                nc.scalar.activation(xn, x_sb[:, tt, :], AF.Identity,
                                     bias=negmr, scale=rstds[:, i:i + 1])
            else:
                nc.vector.tensor_scalar(
                    xn, x_sb[:, tt, :], mvs[:, i, 0:1], rstds[:, i:i + 1],
                    OP.subtract, OP.mult
                )
            tpb = ps_pool.tile([128, 512], f32, tag="misc")
            tp = tpb.bitcast(bf16)[:, 0:C]
            for j in range(3):
                nc.tensor.transpose(
                    tp[:, j * 128:(j + 1) * 128],
                    xn[:, j * 128:(j + 1) * 128], ident,
                )
            dst = xn_ch[:, 0:3, tt * 128:(tt + 1) * 128]
            src = tp.rearrange("p (j t) -> p j t", j=3)
            if xn_drain == "dve":
                nc.vector.tensor_copy(dst, src)
            else:
                nc.scalar.activation(dst, src, AF.Copy)

        # ============ attention + IRB (software-pipelined) ============
        with tc.tile_pool(name="attn_acts", bufs=1) as apool, \
             tc.tile_pool(name="h2_pool", bufs=2) as h2_pool, \
             tc.tile_pool(name="out_pool", bufs=4) as out_pool:
            xn1_ch = apool.tile([128, 4, T], fp8)
            nc.vector.memset(xn1_ch[:, 3, :], 0.0)
            q_sb = apool.tile([96, HEADS, T], bf16)
            k_sb = apool.tile([96, HEADS, T], bf16)
            v_sb = apool.tile([128, NT, HEADS, VP], fp8)
            o_norm = apool.tile([128, NT, HEADS, HD], bf16)
            o_ch = apool.tile([128, BPC, 4, N], fp8)
            nc.vector.memset(o_ch[:, :, 3, :], 0.0)
            nc.vector.memset(v_sb[:, :, :, HD:HD + 1], 1.0)
            nc.vector.memset(v_sb[:, :, :, HD + 1:VP], 0.0)

            with tc.tile_pool(name="ln1", bufs=3) as ln_pool, \
                 tc.tile_pool(name="ln1_ps", bufs=3, space="PSUM") as lnps_pool:
                for g in range(4):
                    tts = list(range(g * 4, g * 4 + 4))
                    mvs, rstds = ln_stats(tts, ln_pool)
                    for i, tt in enumerate(tts):
                        ln_tile(xn1_ch, tt, i, mvs, rstds, ln_pool,
                                lnps_pool)

            nc.vector.memset(h1w_a.bitcast(f32), 0.0)
            nc.vector.memset(h1w_b.bitcast(f32), 0.0)


            def qk_emit(pool, tag, io, h, tk, width):
                dst = q_sb if io == 0 else k_sb
                co = io * C + h * HD
                if pool is None:
                    ps = misc_tile()[0:96, 0:width]
                else:
                    ps = pool.tile([96, width], f32, tag=tag)
                for half in range(width // 512):
                    tsl = slice(tk * width + half * 512,
                                tk * width + half * 512 + 512)
                    psl = slice(half * 512, half * 512 + 512)
                    nc.tensor.matmul(
                        ps[:, psl], wqk_sb[:, 0:2, co:co + HD],
                        xn1_ch[:, 0:2, tsl],
                        start=True, stop=False, perf_mode=DR,
                    )
                    nc.tensor.matmul(
                        ps[:, psl], wqk_sb[:, 2:4, co:co + HD],
                        xn1_ch[:, 2:4, tsl],
                        start=False, stop=True, perf_mode=DR,
                    )
                nc.scalar.activation(
                    dst[:, h, tk * width:(tk + 1) * width], ps, AF.Identity,
                    bias=bqk_sb[:, io * 4 + h: io * 4 + h + 1],
                    scale=1.0 / SQ["qk"],
                )

            def v_emit(pool, tag, tt):
                if pool is None:
                    ps = misc_tile()[:, 0:C]
                else:
                    ps = pool.tile([128, C], f32, tag=tag)
                tsl = slice(tt * 128, (tt + 1) * 128)
                nc.tensor.matmul(
                    ps, xn1_ch[:, 0:2, tsl], wv_sb[:, 0:2, :],
                    start=True, stop=False, perf_mode=DR,
                )
                nc.tensor.matmul(ps, xn1_ch[:, 2:4, tsl], wv_sb[:, 2:4, :],
                                 start=False, stop=not HAS_BIAS,
                                 perf_mode=DR)
                if HAS_BIAS:
                    nc.tensor.matmul(ps, ones8, bv_sb, start=False, stop=True)
                nc.vector.tensor_scalar(
                    v_sb[:, tt, :, 0:HD],
                    ps.rearrange("p (h e) -> p h e", h=HEADS),
                    1.0 / SQ["v"], None, OP.mult,
                )


            with tc.tile_pool(name="qk_ps", bufs=2, space="PSUM") as qk_ps, \
                 tc.tile_pool(name="v_ps", bufs=2, space="PSUM") as v_ps:
                for io in range(2):
                    for h in range(HEADS):
                        qk_emit(qk_ps, "qk", io, h, 0, 1024)
                for tt in range(8):
                    v_emit(v_ps, "v", tt)

            with tc.tile_pool(name="misc_ps", bufs=2, space="PSUM") as misc_ps, \
                 tc.tile_pool(name="pt_pool", bufs=2) as pt_pool, \
                 tc.tile_pool(name="ou_pool", bufs=6) as ou_pool, \
                 tc.tile_pool(name="r_pool", bufs=4) as r_pool, \
                 tc.tile_pool(name="ln2", bufs=3) as ln2_pool:

                def misc_tile():
                    mt_ = misc_ps.tile([128, 512], f32, tag="misc")
                    return mt_

                def norm_emit(b, h, o_un):
                    tp8b = misc_tile()
                    tp8 = tp8b.bitcast(bf16)[:, 0:8 * (HD + 2)] \
                        .rearrange("p (a e) -> p a e", a=8)
                    for ns in range(8):
                        nc.tensor.transpose(
                            tp8[:, ns, 0:HD + 1],
                            o_un[0:HD + 1, ns * 128:(ns + 1) * 128],
                            ident[0:HD + 1, 0:HD + 1],
                        )
                    r8 = r_pool.tile([128, 8], f32, tag="r")
                    nc.vector.reciprocal(r8, tp8[:, :, HD])
                    for ns in range(8):
                        nc.vector.tensor_scalar(
                            o_norm[:, b * 8 + ns, h, :], tp8[:, ns, 0:HD],
                            r8[:, ns:ns + 1], None, OP.mult,
                        )

                def attn_loop(b, filler, st_ps, o_ps, defer=None):
                    def pop(k):
                        for _ in range(k):
                            if filler:
                                filler.pop(0)()
                    for h in range(HEADS):
                        o_psum = o_ps.tile([VP, N], f32, tag="o")
                        for mp in range(4):
                            pt2 = pt_pool.tile([128, 2, N], fp8, tag="pt")
                            for i in range(2):
                                mt = 2 * mp + i
                                st = st_ps.tile([128, N], f32, tag="st")
                                for cn in range(2):
                                    nc.tensor.matmul(
                                        st[:, cn * 512:(cn + 1) * 512],
                                        k_sb[:, h, b * N + mt * 128:
                                             b * N + (mt + 1) * 128],
                                        q_sb[:, h, b * N + cn * 512:
                                             b * N + (cn + 1) * 512],
                                        start=True, stop=True,
                                    )
                                nc.scalar.activation(pt2[:, i, :], st, AF.Exp)
                                pop(1)
                            vp = v_sb[:, b * 8 + 2 * mp: b * 8 + 2 * mp + 2,
                                      h, :]
                            for cn in range(2):
                                nc.tensor.matmul(
                                    o_psum[:, cn * 512:(cn + 1) * 512],
                                    vp, pt2[:, :, cn * 512:(cn + 1) * 512],
                                    start=(mp == 0), stop=(mp == 3),
                                    perf_mode=DR, skip_group_check=True,
                                )
                            pop(1)
                        o_un = ou_pool.tile([VP, N], bf16, tag="ou")
                        nc.vector.tensor_copy(o_un, o_psum)
                        if defer is not None:
                            defer.append((h, o_un))
                        else:
                            norm_emit(b, h, o_un)
                        pop(2)

                def och_proj_emit(b, tl):
                    tt = b * 8 + tl
                    otb = misc_tile()
                    ot = otb.bitcast(bf16)[:, 0:C]
                    ov = o_norm[:, tt, :, :].rearrange("p h e -> p (h e)")
                    for j in range(3):
                        nc.tensor.transpose(
                            ot[:, j * 128:(j + 1) * 128],
                            ov[:, j * 128:(j + 1) * 128], ident,
                        )
                    nc.scalar.activation(
                        o_ch[:, b, 0:3, tl * 128:(tl + 1) * 128],
                        ot.rearrange("p (j t) -> p j t", j=3),
                        AF.Copy,
                    )
                    psb = misc_tile()
                    ps = psb[:, 0:C]
                    tsl = slice(tl * 128, (tl + 1) * 128)
                    nc.tensor.matmul(
                        ps, o_ch[:, b, 0:2, tsl], wp_sb[:, 0:2, :],
                        start=True, stop=False, perf_mode=DR,
                    )
                    nc.tensor.matmul(ps, o_ch[:, b, 2:4, tsl],
                                     wp_sb[:, 2:4, :],
                                     start=False, stop=not HAS_BIAS,
                                     perf_mode=DR)
                    if HAS_BIAS:
                        nc.tensor.matmul(ps, ones8, bp_sb,
                                         start=False, stop=True)
                    nc.vector.scalar_tensor_tensor(
                        x_sb[:, tt, :], ps, 1.0 / SQ["p"], x_sb[:, tt, :],
                        OP.mult, OP.add,
                    )

                h2_tiles = {}

                def fc1_emit(b, m, half, pool, tag):
                    # fc1 for 512 tokens (16 image rows) -> hswish -> window
                    h1w = h1w_bufs[b]
                    msl = slice(m * 128, (m + 1) * 128)
                    c0 = half * 512
                    if tag == "misc":
                        psb = misc_tile()
                        ps = psb[:, 0:512]
                    else:
                        ps = pool.tile([128, 512], f32, tag=tag)
                    csl = slice(b * N + c0, b * N + c0 + 512)
                    nc.tensor.matmul(
                        ps, wf1_sb[:, 0:2, msl], xn2_ch[:, 0:2, csl],
                        start=True, stop=False, perf_mode=DR,
                    )
                    nc.tensor.matmul(
                        ps, wf1_sb[:, 2:4, msl], xn2_ch[:, 2:4, csl],
                        start=False, stop=True, perf_mode=DR,
                    )
                    lo = HOFF + (1 + 16 * half) * WP
                    h1v = h1w[:, m, lo:lo + 16 * WP] \
                        .rearrange("p (y x) -> p y x", x=WP)[:, :, 0:32]
                    nc.vector._custom_dve(
                        HSWISH6Q, out=h1v,
                        in0=ps.rearrange("p (y x) -> p y x", x=32),
                        in1=three, s0=bf13_sb[:, m:m + 1],
                        s1=1.0 / SQ["f1"], imm2=6.0,
                    )

                def dw_taps(dps_seg, h1w, m, base, u0, u1):
                    for p, (ta, tb) in enumerate(DW_PAIRS):
                        soa = tap_off(ta) + base
                        nc.tensor.matmul(
                            dps_seg[:, u0:u1],
                            wdgp_sb[:, m, 2 * p:2 * p + 2, :],
                            pair_ap(h1w[:, m, :], tap_off(tb) - tap_off(ta),
                                    soa + u0, soa + u1),
                            start=(p == 0), stop=False,
                            perf_mode=DR, skip_group_check=True,
                        )
                    for si, t in enumerate(DW_SINGLES):
                        so = tap_off(t) + base
                        nc.tensor.matmul(
                            dps_seg[:, u0:u1], wdgs_sb[:, m, si, :],
                            h1w[:, m, so + u0:so + u1],
                            start=False, stop=(si == 4),
                            skip_group_check=True,
                        )

                def hswish2(h2, m, half2, dps, nrows):
                    # half2: 0 or 1 (which 16-row half of the image)
                    nc.vector._custom_dve(
                        HSWISH6Q,
                        out=h2[:, m, half2 * 512:half2 * 512 + nrows * 32]
                        .rearrange("p (y x) -> p y x", x=32),
                        in0=dps.rearrange(
                            "p (y x) -> p y x", x=WP)[:, 0:nrows, 0:32],
                        in1=three, s0=bdw3_sb[:, m:m + 1],
                        s1=1.0 / SQ["dw"], imm2=6.0,
                    )

                def fc1dw_emit(b, m, dwpool):
                    # filler path: fc1 halves via misc, dw via two 2-bank psums
                    fc1_emit(b, m, 0, None, "misc")
                    fc1_emit(b, m, 1, None, "misc")
                    h1w = h1w_bufs[b]
                    if m == 0:
                        h2_tiles[b] = h2_pool.tile(
                            [128, 12, 1024], fp8, tag="h2", name="h2f")
                    h2 = h2_tiles[b]
                    for half2 in range(2):
                        dps = dwpool.tile([128, 640], f32, tag="dwp")
                        base = half2 * 16 * WP
                        for u0, u1 in SEGS2A:
                            dw_taps(dps, h1w, m, base, u0, u1)
                        hswish2(h2, m, half2, dps, 16)

                def fc2_emit(b, tl, pool, tag):
                    tg = b * 8 + tl
                    h2 = h2_tiles[b]
                    if tag == "misc":
                        psb = misc_tile()
                        ps = psb[:, 0:C]
                    else:
                        ps = pool.tile([128, C], f32, tag=tag)
                    for mi in range(6):
                        nc.tensor.matmul(
                            ps, h2[:, 2 * mi:2 * mi + 2,
                                   tl * 128:(tl + 1) * 128],
                            wf2_sb[:, 2 * mi:2 * mi + 2, :],
                            start=(mi == 0),
                            stop=(mi == 5 and not HAS_BIAS),
                            perf_mode=DR,
                        )
                    if HAS_BIAS:
                        nc.tensor.matmul(ps, ones8, bf2_sb,
                                         start=False, stop=True)
                    ot = out_pool.tile([128, C], f32, tag="out")
                    nc.vector.scalar_tensor_tensor(
                        ot, ps, 1.0 / SQ["f2"], x_sb[:, tg, :],
                        OP.mult, OP.add,
                    )
                    nc.sync.dma_start(
                        out=d["out"][b, tl * 128:(tl + 1) * 128, :],
                        in_=ot,
                    )

                def ln2_emit(b, g2=None):
                    gs = [g2] if g2 is not None else [0, 1]
                    for g in gs:
                        btts = list(range(b * 8 + g * 4, b * 8 + g * 4 + 4))
                        mvs, rstds = ln_stats(btts, ln2_pool)
                        for i, tt in enumerate(btts):
                            ln_tile(xn2_ch, tt, i, mvs, rstds, ln2_pool,
                                    misc_ps, xn_drain="dve")

                # ---- batch 0 attention: dense, double-buffered scores ----
                d0 = []
                with tc.tile_pool(name="st2_ps", bufs=2,
                                  space="PSUM") as st2_ps, \
                     tc.tile_pool(name="o0_ps", bufs=1,
                                  space="PSUM") as o0_ps:
                    qkv1 = []
                    for io in range(2):
                        for h in range(HEADS):
                            for tk in range(2, 4):
                                qkv1.append(lambda io=io, h=h, tk=tk:
                                            qk_emit(None, "", io, h, tk, 512))
                    for tt in range(8, NT):
                        qkv1.append(lambda tt=tt: v_emit(None, "", tt))
                    attn_loop(0, qkv1, st2_ps, o0_ps, defer=d0)
                    while qkv1:
                        qkv1.pop(0)()

                # ---- batch 1 attention; batch 0 proj/LN2/IRB as filler ----
                with tc.tile_pool(name="st1_ps", bufs=1,
                                  space="PSUM") as st1_ps, \
                     tc.tile_pool(name="o1_ps", bufs=1,
                                  space="PSUM") as o1_ps, \
                     tc.tile_pool(name="dwf_ps", bufs=1,
                                  space="PSUM") as dwf_ps:
                    flr = [lambda h=h, ou=ou: norm_emit(0, h, ou)
                           for h, ou in d0]
                    flr += [lambda tl=tl: och_proj_emit(0, tl)
                            for tl in range(8)]
                    flr.append(lambda: ln2_emit(0))
                    for m in range(12):
                        flr.append(lambda m=m: fc1dw_emit(0, m, dwf_ps))
                    for tl in range(8):
                        flr.append(lambda tl=tl: fc2_emit(0, tl, None,
                                                          "misc"))
                    attn_loop(1, flr, st1_ps, o1_ps)
                    # batch 1 proj interleaved with leftover batch-0 IRB
                    for tl in range(8):
                        och_proj_emit(1, tl)
                        if flr:
                            flr.pop(0)()
                        if tl == 3:
                            ln2_emit(1, 0)
                    ln2_emit(1, 1)
                    while flr:
                        flr.pop(0)()

                # batch-1 IRB with a double-buffered dw psum (st/o pools
                # closed free the banks): fc1+dw per chunk, then fc2
                with tc.tile_pool(name="dwt_ps", bufs=2,
                                  space="PSUM") as dwt_ps:
                    for m in range(12):
                        fc1dw_emit(1, m, dwt_ps)
                    for tl in range(8):
                        fc2_emit(1, tl, None, "misc")


def declare_tensors(nc, scales, has_bias):
    d = {"scales": scales, "has_bias": has_bias}
    d["x"] = nc.dram_tensor("x", [BPC, N, C], bf16, kind="ExternalInput").ap()
    d["wqk"] = nc.dram_tensor("wqk", [4, 128, 2 * C], fp8, kind="ExternalInput").ap()
    d["bqk"] = nc.dram_tensor("bqk", [96, 8], f32, kind="ExternalInput").ap()
    d["wv"] = nc.dram_tensor("wv", [4, 128, C], fp8, kind="ExternalInput").ap()
    d["bv"] = nc.dram_tensor("bv", [1, C], fp8, kind="ExternalInput").ap()
    d["wp"] = nc.dram_tensor("wp", [4, 128, C], fp8, kind="ExternalInput").ap()
    d["bp"] = nc.dram_tensor("bp", [1, C], fp8, kind="ExternalInput").ap()
    d["wf1"] = nc.dram_tensor("wf1", [4, 128, HID], fp8, kind="ExternalInput").ap()
    d["bf13"] = nc.dram_tensor("bf13", [128, 12], f32, kind="ExternalInput").ap()
    d["wdgp"] = nc.dram_tensor("wdgp", [12, 4, 128, 128], fp8,
                               kind="ExternalInput").ap()
    d["wdgs"] = nc.dram_tensor("wdgs", [12, 5, 128, 128], fp8,
                               kind="ExternalInput").ap()
    d["bdw3"] = nc.dram_tensor("bdw3", [128, 12], f32, kind="ExternalInput").ap()
    d["wf2"] = nc.dram_tensor("wf2", [12, 128, C], fp8, kind="ExternalInput").ap()
    d["bf2"] = nc.dram_tensor("bf2", [1, C], fp8, kind="ExternalInput").ap()
    d["out"] = nc.dram_tensor("out", [BPC, N, C], f32, kind="ExternalOutput").ap()
    return d


@functools.lru_cache(maxsize=1)
def build_program(scale_items, has_bias=False, num_devices=NCORES):
    scales = dict(scale_items)
    nc = bacc.Bacc("TRN2", target_bir_lowering=False, debug=False,
                   num_devices=num_devices)
    d = declare_tensors(nc, scales, has_bias)
    with tile.TileContext(nc) as tc:
        emit_kernel(nc, tc, d)
    nc.compile()
    return nc


def _scale_for(w):
    m = float(np.abs(w).max())
    return 224.0 / m if m > 0 else 1.0


def prep_weights(inputs):
    """Host-side packing: transposes, LN folds, fp8 quantization + scales."""
    g1 = np.asarray(inputs["ln1_g"], np.float32)
    b1 = np.asarray(inputs["ln1_b"], np.float32)
    g2 = np.asarray(inputs["ln2_g"], np.float32)
    b2 = np.asarray(inputs["ln2_b"], np.float32)
    Wqkv = np.asarray(inputs["Wqkv"], np.float32)
    Wproj = np.asarray(inputs["Wproj"], np.float32)
    bproj = np.asarray(inputs["bproj"], np.float32)
    Wfc1 = np.asarray(inputs["Wfc1"], np.float32)[:, :, 0, 0]
    bfc1 = np.asarray(inputs["bfc1"], np.float32)
    Wdw = np.asarray(inputs["Wdw"], np.float32)[:, 0].reshape(HID, 9)
    bdw = np.asarray(inputs["bdw"], np.float32)
    Wfc2 = np.asarray(inputs["Wfc2"], np.float32)[:, :, 0, 0]
    bfc2 = np.asarray(inputs["bfc2"], np.float32)

    W3 = Wqkv.reshape(HEADS, 3, HD, C)      # out channel o = h*288 + s*96 + d
    scale = float(HD) ** -0.5
    Wq = W3[:, 0].reshape(HEADS * HD, C)
    Wk = W3[:, 1].reshape(HEADS * HD, C)
    Wv = W3[:, 2].reshape(HEADS * HD, C)

    wqk_full = np.concatenate([Wq * g1[None, :] * scale, Wk * g1[None, :]], 0)
    wv_full = Wv * g1[None, :]
    wf1_full = Wfc1 * g2[None, :]
    wdw_full = Wdw / 6.0
    wf2_full = Wfc2 / 6.0

    sc = {
        "qk": _scale_for(wqk_full), "v": _scale_for(wv_full),
        "p": _scale_for(Wproj), "f1": _scale_for(wf1_full),
        "dw": _scale_for(wdw_full), "f2": _scale_for(wf2_full),
    }

    d = {}
    def plane4(wt, ncols):
        w = np.zeros((4, 128, ncols), np.float32)
        w[0:3] = wt.T.reshape(3, 128, ncols)
        return w.astype(nf8)

    d["wqk"] = plane4(wqk_full * sc["qk"], 2 * C)
    d["bqk"] = np.ascontiguousarray(np.concatenate(
        [((Wq @ b1) * scale).reshape(HEADS, HD).T,
         (Wk @ b1).reshape(HEADS, HD).T], 1)).astype(np.float32)
    d["wv"] = plane4(wv_full * sc["v"], C)
    d["bv"] = ((Wv @ b1) * sc["v"])[None, :].astype(nf8)
    d["wp"] = plane4(Wproj * sc["p"], C)
    d["bp"] = (bproj * sc["p"])[None, :].astype(nf8)
    d["wf1"] = plane4(wf1_full * sc["f1"], HID)
    d["bf13"] = np.ascontiguousarray(
        (bfc1 + Wfc1 @ b2 + 3.0).reshape(12, 128).T).astype(np.float32)
    wd = wdw_full * sc["dw"]
    wdgp = np.zeros((12, 4, 128, 128), np.float32)
    wdgs = np.zeros((12, 5, 128, 128), np.float32)
    ii = np.arange(128)
    for m in range(12):
        for p, (ta, tb) in enumerate(DW_PAIRS):
            wdgp[m, 2 * p + 0, ii, ii] = wd[m * 128 + ii, ta]
            wdgp[m, 2 * p + 1, ii, ii] = wd[m * 128 + ii, tb]
        for si, t in enumerate(DW_SINGLES):
            wdgs[m, si, ii, ii] = wd[m * 128 + ii, t]
    d["wdgp"] = wdgp.astype(nf8)
    d["wdgs"] = wdgs.astype(nf8)
    d["bdw3"] = np.ascontiguousarray(
        (bdw + 3.0).reshape(12, 128).T).astype(np.float32)
    d["wf2"] = np.ascontiguousarray(
        (wf2_full * sc["f2"]).T.reshape(12, 128, C)).astype(nf8)
    d["bf2"] = (bfc2 * sc["f2"])[None, :].astype(nf8)
    has_bias = any(float(np.abs(v).max()) > 0 for v in
                   (Wv @ b1, bproj, bfc2))
    return d, sc, has_bias


def kernel(**inputs):
    from concourse.bass_utils import run_bass_kernel_spmd

    x = np.asarray(inputs["x"], np.float32)
    wd, sc, has_bias = prep_weights(inputs)
    nc = build_program(tuple(sorted(sc.items())), has_bias)
    in_maps = []
    for c in range(NCORES):
        m = dict(wd)
        m["x"] = np.ascontiguousarray(
            x[c * BPC:(c + 1) * BPC]).astype(nbf)
        in_maps.append(m)
    res = run_bass_kernel_spmd(nc, in_maps, list(range(NCORES)))
    out = np.concatenate([res.results[c]["out"] for c in range(NCORES)], axis=0)
    return out.astype(np.float32)


# revision 42
# speedup vs baseline: 1.0189x; 1.0189x over previous
"""Trainium2 Bass kernel for nn_Block_79680233275670 (dense transformer block).

Reference, for x [16, 1024, 384]:
  x = x + proj(attn(LN1(x)))                               (4 heads, head_dim 96)
  x = x + fc2(hswish(dw3x3(hswish(fc1(LN2(x))))))          (IRB, 32x32 spatial)

Sharding: pure data-parallel over batch B=16 -> 8 cores x 2 batch items.
No collectives. Weights replicated (pre-transposed / LN-folded / fp8-quantized
host-side).

Per-core dataflow (T = 2048 tokens = 2 batches x 1024):
  - x token-major [128, 16, 384] f32 (4 chunked DMAs); residual stream f32
  - LN token-major (bn_stats) -> bf16, PE-transpose -> ACT fp8 cast
  - fp8e4 DoubleRow matmuls (2 k-tiles/pass) for QKV / PV / proj / fc1 / fc2;
    scores q^T k stay bf16
  - per (batch, head): scores St[m,n] = k^T q in a double-buffered PSUM;
    exp on ACT emits fp8 P m-tile pairs = the DoubleRow rhs for PV. An
    appended ones column in v (padded to 112 rows) makes PV also emit
    softmax denominators. O-normalization interleaved per head: DVE drain,
    PE transposes into one [128,8,98] PSUM tile, one recip, per-slice mults
  - proj/fc2 token-major; fp8 dequant scale folded into the
    scalar_tensor_tensor residual add (x += psum*s)
  - IRB: fc1 channel-major fp8; hardswish = ONE custom DVE op
    (min(relu(x*s+b+3),6)*(relu(..)-3), /6 folded into dw / fc2 weights)
    writing fp8 windows directly; depthwise 3x3 on PE over 19-row
    zero-padded windows (40-elem row pitch): 2 DoubleRow tap pairs
    ((-1,dx),(+1,dx) for dx=+-1, 80-elem k-stride satisfies the 16B rule)
    + 5 single taps
"""

import sys
import functools

for _p in ("/opt/trn_rl_repo",):
    if _p not in sys.path:
        sys.path.insert(0, _p)

import numpy as np
import ml_dtypes

import concourse.bass as bass
import concourse.mybir as mybir
import concourse.tile as tile
from concourse import bacc
from concourse.ap import AP
from concourse.masks import make_identity


B, N, C = 16, 1024, 384
HEADS, HD = 4, 96
VP = 112                   # v rows incl. ones col, padded for DR alignment
HID = 1536
NCORES = 8
BPC = B // NCORES          # batches per core
T = BPC * N                # tokens per core
NT = T // 128              # 16 token tiles per core
EPS = 1e-5

f32 = mybir.dt.float32
bf16 = mybir.dt.bfloat16
fp8 = mybir.dt.float8e4
AF = mybir.ActivationFunctionType
OP = mybir.AluOpType
DR = mybir.MatmulPerfMode.DoubleRow
nbf = ml_dtypes.bfloat16
nf8 = ml_dtypes.float8_e4m3

# ---- custom fused hardswish DVE op (registered at import time) ------------
# out = min(relu(in*C1 + C0), 6) * (relu(in*C1 + C0) - 3)
#     = hswish6(in*C1 + (C0-3)) where hswish6(x) = x*clip(x+3,0,6)
# C0 = bias+3 (per-partition AP), C1 = fp8 dequant scale, C2 = 6 (imm2),
# C3 = 3 delivered via in1 (spilled).
import concourse.dve_ops as dve_ops
from concourse.dve_spec import Spec, Src0, C0, C1, C2, C3, relu, minn, lower
from concourse.dve_ops import DveOp, DveOpSpec, _spill_c3_to_src1


def _register_hswish():
    name = "HSWISH6Q_ANT"
    if name in dve_ops._SUB_OPCODE_FOR_NAME:
        for op in dve_ops.OPS:
            if op.name == name:
                return op
    r = relu(Src0 * C1 + C0)
    spec = Spec(
        body=_spill_c3_to_src1(minn(r, C2) * (r - C3)),
        reference=lambda in0, in1, s0, s1, imm2:
        np.minimum(np.maximum(in0 * s1 + s0, 0), imm2)
        * (np.maximum(in0 * s1 + s0, 0) - in1),
    )
    op = DveOp(name, spec, subdim=False, uops_sha={})
    row = dve_ops._CUSTOM_DVE_ROW_BASE + len(dve_ops.OPS)
    assert row < 0x20
    for ver in ("v3", "v4"):
        probe = DveOpSpec(name=name, opcode=row, uops=lower(spec, ver=ver),
                          rd1_en=True)
        op.uops_sha[ver] = probe.sha(ver)
    dve_ops.OPS.append(op)
    dve_ops._SUB_OPCODE_FOR_NAME[name] = row
    dve_ops.CUSTOM_DVE_SPECS[name] = spec
    return op


HSWISH6Q = _register_hswish()

# ---- depthwise-window geometry -------------------------------------------
# 19 rows per window: row 0 and row 18 stay zero (vertical SAME padding),
# rows 1..17 hold 17 image rows (16 outputs + 1 halo). Rows padded to WP=40
# (32 data + 8 zero cols -> horizontal SAME padding), plus HOFF=3 leading
# zeros. All 9 taps share identical geometry:
#   acc[0:AUSE) += w_t * win[so : so+AUSE),  so = HOFF+(dy+1+yh)*WP+dx
# Taps (-1,dx) and (+1,dx) pair into one DoubleRow matmul (k-stride 2*WP=80,
# even offsets for dx=+-1 since HOFF=3).
WP = 40
HOFF = 3
HLEN = HOFF + 34 * WP + 1  # rows: 1 zero + 32 image rows + 1 zero (pad->/4)
ACCL = 32 * WP             # 1280 acc length (full batch image)
AUSE = ACCL - 2            # 1278 initialized prefix
SEGS3 = ((0, 512), (512, 1024), (1024, AUSE))
SEGS2A = ((0, 512), (512, 638))          # half-image variants (2-bank psum)
DW_PAIRS = ((0, 6), (2, 8))      # (dy=-1,dx) + (dy=+1,dx) for dx=-1,+1
DW_SINGLES = (1, 3, 4, 5, 7)


def tap_off(t):
    # window row w holds image row w-1; acc row y reads window row y+dy+1
    dy, dx = t // 3 - 1, t % 3 - 1
    return HOFF + (dy + 1) * WP + dx


def pair_ap(w, delta, lo, hi):
    """[128, 2, hi-lo] view: two copies of w[:, lo:hi] offset by delta elems."""
    v = w[:, lo:hi]
    return AP(v.tensor, v.offset, [list(v.ap[0]), [delta, 2], [1, hi - lo]])


def emit_kernel(nc, tc, d):
    from contextlib import ExitStack

    with ExitStack() as ctx:
        singles = ctx.enter_context(tc.tile_pool(name="singles", bufs=1))

        x_sb = singles.tile([128, NT, C], bf16)  # token-major; becomes x2 in place
        ident = singles.tile([128, 128], bf16)
        make_identity(nc, ident)
        ones8 = singles.tile([1, 128], fp8)
        nc.vector.memset(ones8, 1.0)
        eps_sb = singles.tile([128, 1], f32)
        nc.vector.memset(eps_sb, EPS)
        three = singles.tile([128, 1], f32)
        nc.vector.memset(three, 3.0)

        wqk_sb = singles.tile([128, 4, 2 * C], fp8)
        bqk_sb = singles.tile([96, 8], f32)
        wv_sb = singles.tile([128, 4, C], fp8)
        bv_sb = singles.tile([1, C], fp8)
        wp_sb = singles.tile([128, 4, C], fp8)
        bp_sb = singles.tile([1, C], fp8)
        wf1_sb = singles.tile([128, 4, HID], fp8)
        bf13_sb = singles.tile([128, 12], f32)
        wdgp_sb = singles.tile([128, 12, 4, 128], fp8)
        wdgs_sb = singles.tile([128, 12, 5, 128], fp8)
        bdw3_sb = singles.tile([128, 12], f32)
        wf2_sb = singles.tile([128, 12, C], fp8)
        bf2_sb = singles.tile([1, C], fp8)

        xn2_ch = singles.tile([128, 4, T], fp8)
        nc.vector.memset(xn2_ch[:, 3, :], 0.0)
        h1w_a = singles.tile([128, 12, HLEN], fp8)
        h1w_b = singles.tile([128, 12, HLEN], fp8)
        h1w_bufs = [h1w_a, h1w_b]

        # x[b, i*128+p, c] -> x_sb[p, b*8+i, c], 8 chunks on 2 queues so
        # LN1 starts early and the load runs on two DMA paths
        for ch in range(8):
            eng = nc.sync if ch % 2 == 0 else nc.scalar
            eng.dma_start(
                out=x_sb[:, ch * 2:(ch + 1) * 2, :],
                in_=d["x"].rearrange("b (i p) c -> p (b i) c", p=128)
                [:, ch * 2:(ch + 1) * 2, :],
            )
        for name, dst in (("wqk", wqk_sb), ("wv", wv_sb), ("wp", wp_sb)):
            nc.sync.dma_start(out=dst, in_=d[name].rearrange("k p m -> p k m"))
        for name, dst in (("bqk", bqk_sb), ("bv", bv_sb), ("bp", bp_sb),
                          ("bf13", bf13_sb), ("bdw3", bdw3_sb),
                          ("bf2", bf2_sb)):
            nc.sync.dma_start(out=dst, in_=d[name])
        nc.sync.dma_start(out=wf1_sb, in_=d["wf1"].rearrange("k p m -> p k m"))
        nc.sync.dma_start(out=wf2_sb, in_=d["wf2"].rearrange("k p m -> p k m"))
        nc.scalar.dma_start(
            out=wdgp_sb, in_=d["wdgp"].rearrange("m g c j -> c m g j"))
        nc.scalar.dma_start(
            out=wdgs_sb, in_=d["wdgs"].rearrange("m s c j -> c m s j"))

        SQ = d["scales"]  # dict of python floats
        HAS_BIAS = d["has_bias"]

        def ln_stats(tts, ln_pool):
            # one Sqrt + one reciprocal for the whole tile group
            G = len(tts)
            mvs = ln_pool.tile([128, 8, 2], f32, tag="ln_mvs")
            for i, tt in enumerate(tts):
                stats = ln_pool.tile([128, 6], f32, tag="ln_stats")
                nc.vector.bn_stats(stats, x_sb[:, tt, :])
                nc.vector.bn_aggr(mvs[:, i, :], stats)
            stds = ln_pool.tile([128, 8], f32, tag="ln_stds")
            nc.scalar.activation(stds[:, 0:G], mvs[:, 0:G, 1], AF.Sqrt,
                                 bias=eps_sb)
            rstds = ln_pool.tile([128, 8], f32, tag="ln_rstds")
            nc.vector.reciprocal(rstds[:, 0:G], stds[:, 0:G])
            return mvs, rstds

        def ln_tile(xn_ch, tt, i, mvs, rstds, ln_pool, ps_pool,
                    xn_drain=None, affine="dve"):
            xn = ln_pool.tile([128, C], bf16, tag="ln_xn")
            if affine == "act":
                negmr = ln_pool.tile([128, 1], f32, tag="ln_negmr")
                nc.vector.tensor_scalar(
                    negmr, mvs[:, i, 0:1], -1.0, rstds[:, i:i + 1],
                    OP.mult, OP.mult
                )
                nc.scalar.activation(xn, x_sb[:, tt, :], AF.Identity,
                                     bias=negmr, scale=rstds[:, i:i + 1])
            else:
                nc.vector.tensor_scalar(
                    xn, x_sb[:, tt, :], mvs[:, i, 0:1], rstds[:, i:i + 1],
                    OP.subtract, OP.mult
                )
            tpb = ps_pool.tile([128, 512], f32, tag="misc")
            tp = tpb.bitcast(bf16)[:, 0:C]
            for j in range(3):
                nc.tensor.transpose(
                    tp[:, j * 128:(j + 1) * 128],
                    xn[:, j * 128:(j + 1) * 128], ident,
                )
            dst = xn_ch[:, 0:3, tt * 128:(tt + 1) * 128]
            src = tp.rearrange("p (j t) -> p j t", j=3)
            if xn_drain == "dve":
                nc.vector.tensor_copy(dst, src)
            else:
                nc.scalar.activation(dst, src, AF.Copy)

        # ============ attention + IRB (software-pipelined) ============
        with tc.tile_pool(name="attn_acts", bufs=1) as apool, \
             tc.tile_pool(name="h2_pool", bufs=2) as h2_pool, \
             tc.tile_pool(name="out_pool", bufs=4) as out_pool:
            xn1_ch = apool.tile([128, 4, T], fp8)
            nc.vector.memset(xn1_ch[:, 3, :], 0.0)
            q_sb = apool.tile([96, HEADS, T], bf16)
            k_sb = apool.tile([96, HEADS, T], bf16)
            v_sb = apool.tile([128, NT, HEADS, VP], fp8)
            o_norm = apool.tile([128, NT, HEADS, HD], bf16)
            o_ch = apool.tile([128, BPC, 4, N], fp8)
            nc.vector.memset(o_ch[:, :, 3, :], 0.0)
            nc.vector.memset(v_sb[:, :, :, HD:HD + 1], 1.0)
            nc.vector.memset(v_sb[:, :, :, HD + 1:VP], 0.0)

            with tc.tile_pool(name="ln1", bufs=3) as ln_pool, \
                 tc.tile_pool(name="ln1_ps", bufs=3, space="PSUM") as lnps_pool:
                for g in range(4):
                    tts = list(range(g * 4, g * 4 + 4))
                    mvs, rstds = ln_stats(tts, ln_pool)
                    for i, tt in enumerate(tts):
                        ln_tile(xn1_ch, tt, i, mvs, rstds, ln_pool,
                                lnps_pool)

            nc.vector.memset(h1w_a.bitcast(f32), 0.0)
            nc.vector.memset(h1w_b.bitcast(f32), 0.0)


            def qk_emit(pool, tag, io, h, tk, width):
                dst = q_sb if io == 0 else k_sb
                co = io * C + h * HD
                if pool is None:
                    ps = misc_tile()[0:96, 0:width]
                else:
                    ps = pool.tile([96, width], f32, tag=tag)
                for half in range(width // 512):
                    tsl = slice(tk * width + half * 512,
                                tk * width + half * 512 + 512)
                    psl = slice(half * 512, half * 512 + 512)
                    nc.tensor.matmul(
                        ps[:, psl], wqk_sb[:, 0:2, co:co + HD],
                        xn1_ch[:, 0:2, tsl],
                        start=True, stop=False, perf_mode=DR,
                    )
                    nc.tensor.matmul(
                        ps[:, psl], wqk_sb[:, 2:4, co:co + HD],
                        xn1_ch[:, 2:4, tsl],
                        start=False, stop=True, perf_mode=DR,
                    )
                nc.scalar.activation(
                    dst[:, h, tk * width:(tk + 1) * width], ps, AF.Identity,
                    bias=bqk_sb[:, io * 4 + h: io * 4 + h + 1],
                    scale=1.0 / SQ["qk"],
                )

            def v_emit(pool, tag, tt):
                if pool is None:
                    ps = misc_tile()[:, 0:C]
                else:
                    ps = pool.tile([128, C], f32, tag=tag)
                tsl = slice(tt * 128, (tt + 1) * 128)
                nc.tensor.matmul(
                    ps, xn1_ch[:, 0:2, tsl], wv_sb[:, 0:2, :],
                    start=True, stop=False, perf_mode=DR,
                )
                nc.tensor.matmul(ps, xn1_ch[:, 2:4, tsl], wv_sb[:, 2:4, :],
                                 start=False, stop=not HAS_BIAS,
                                 perf_mode=DR)
                if HAS_BIAS:
                    nc.tensor.matmul(ps, ones8, bv_sb, start=False, stop=True)
                nc.vector.tensor_scalar(
                    v_sb[:, tt, :, 0:HD],
                    ps.rearrange("p (h e) -> p h e", h=HEADS),
                    1.0 / SQ["v"], None, OP.mult,
                )


            with tc.tile_pool(name="qk_ps", bufs=2, space="PSUM") as qk_ps, \
                 tc.tile_pool(name="v_ps", bufs=2, space="PSUM") as v_ps:
                for io in range(2):
                    for h in range(HEADS):
                        for cn in range(2):
                            qk_emit(qk_ps, "qk", io, h, cn, 1024)
                for tt in range(NT):
                    v_emit(v_ps, "v", tt)

            with tc.tile_pool(name="misc_ps", bufs=2, space="PSUM") as misc_ps, \
                 tc.tile_pool(name="pt_pool", bufs=2) as pt_pool, \
                 tc.tile_pool(name="ou_pool", bufs=6) as ou_pool, \
                 tc.tile_pool(name="r_pool", bufs=4) as r_pool, \
                 tc.tile_pool(name="ln2", bufs=3) as ln2_pool:

                def misc_tile():
                    mt_ = misc_ps.tile([128, 512], f32, tag="misc")
                    return mt_

                def norm_emit(b, h, o_un):
                    tp8b = misc_tile()
                    tp8 = tp8b.bitcast(bf16)[:, 0:8 * (HD + 2)] \
                        .rearrange("p (a e) -> p a e", a=8)
                    for ns in range(8):
                        nc.tensor.transpose(
                            tp8[:, ns, 0:HD + 1],
                            o_un[0:HD + 1, ns * 128:(ns + 1) * 128],
                            ident[0:HD + 1, 0:HD + 1],
                        )
                    r8 = r_pool.tile([128, 8], f32, tag="r")
                    nc.vector.reciprocal(r8, tp8[:, :, HD])
                    for ns in range(8):
                        nc.vector.tensor_scalar(
                            o_norm[:, b * 8 + ns, h, :], tp8[:, ns, 0:HD],
                            r8[:, ns:ns + 1], None, OP.mult,
                        )

                def attn_loop(b, filler, st_ps, o_ps, defer=None):
                    def pop(k):
                        for _ in range(k):
                            if filler:
                                filler.pop(0)()
                    for h in range(HEADS):
                        o_psum = o_ps.tile([VP, N], f32, tag="o")
                        for mp in range(4):
                            pt2 = pt_pool.tile([128, 2, N], fp8, tag="pt")
                            for i in range(2):
                                mt = 2 * mp + i
                                st = st_ps.tile([128, N], f32, tag="st")
                                for cn in range(2):
                                    nc.tensor.matmul(
                                        st[:, cn * 512:(cn + 1) * 512],
                                        k_sb[:, h, b * N + mt * 128:
                                             b * N + (mt + 1) * 128],
                                        q_sb[:, h, b * N + cn * 512:
                                             b * N + (cn + 1) * 512],
                                        start=True, stop=True,
                                    )
                                nc.scalar.activation(pt2[:, i, :], st, AF.Exp)
                                pop(1)
                            vp = v_sb[:, b * 8 + 2 * mp: b * 8 + 2 * mp + 2,
                                      h, :]
                            for cn in range(2):
                                nc.tensor.matmul(
                                    o_psum[:, cn * 512:(cn + 1) * 512],
                                    vp, pt2[:, :, cn * 512:(cn + 1) * 512],
                                    start=(mp == 0), stop=(mp == 3),
                                    perf_mode=DR, skip_group_check=True,
                                )
                            pop(1)
                        o_un = ou_pool.tile([VP, N], bf16, tag="ou")
                        nc.vector.tensor_copy(o_un, o_psum)
                        if defer is not None:
                            defer.append((h, o_un))
                        else:
                            norm_emit(b, h, o_un)
                        pop(2)

                def och_proj_emit(b, tl, pjpool=None):
                    tt = b * 8 + tl
                    otb = misc_tile()
                    ot = otb.bitcast(bf16)[:, 0:C]
                    ov = o_norm[:, tt, :, :].rearrange("p h e -> p (h e)")
                    for j in range(3):
                        nc.tensor.transpose(
                            ot[:, j * 128:(j + 1) * 128],
                            ov[:, j * 128:(j + 1) * 128], ident,
                        )
                    nc.scalar.activation(
                        o_ch[:, b, 0:3, tl * 128:(tl + 1) * 128],
                        ot.rearrange("p (j t) -> p j t", j=3),
                        AF.Copy,
                    )
                    if pjpool is not None:
                        psb = pjpool.tile([128, 512], f32, tag="pj2")
                    else:
                        psb = misc_tile()
                    ps = psb[:, 0:C]
                    tsl = slice(tl * 128, (tl + 1) * 128)
                    nc.tensor.matmul(
                        ps, o_ch[:, b, 0:2, tsl], wp_sb[:, 0:2, :],
                        start=True, stop=False, perf_mode=DR,
                    )
                    nc.tensor.matmul(ps, o_ch[:, b, 2:4, tsl],
                                     wp_sb[:, 2:4, :],
                                     start=False, stop=not HAS_BIAS,
                                     perf_mode=DR)
                    if HAS_BIAS:
                        nc.tensor.matmul(ps, ones8, bp_sb,
                                         start=False, stop=True)
                    nc.vector.scalar_tensor_tensor(
                        x_sb[:, tt, :], ps, 1.0 / SQ["p"], x_sb[:, tt, :],
                        OP.mult, OP.add,
                    )

                h2_tiles = {}

                def fc1_emit(b, m, half, pool, tag):
                    # fc1 for 512 tokens (16 image rows) -> hswish -> window
                    h1w = h1w_bufs[b]
                    msl = slice(m * 128, (m + 1) * 128)
                    c0 = half * 512
                    if tag == "misc":
                        psb = misc_tile()
                        ps = psb[:, 0:512]
                    else:
                        ps = pool.tile([128, 512], f32, tag=tag)
                    csl = slice(b * N + c0, b * N + c0 + 512)
                    nc.tensor.matmul(
                        ps, wf1_sb[:, 0:2, msl], xn2_ch[:, 0:2, csl],
                        start=True, stop=False, perf_mode=DR,
                    )
                    nc.tensor.matmul(
                        ps, wf1_sb[:, 2:4, msl], xn2_ch[:, 2:4, csl],
                        start=False, stop=True, perf_mode=DR,
                    )
                    lo = HOFF + (1 + 16 * half) * WP
                    h1v = h1w[:, m, lo:lo + 16 * WP] \
                        .rearrange("p (y x) -> p y x", x=WP)[:, :, 0:32]
                    nc.vector._custom_dve(
                        HSWISH6Q, out=h1v,
                        in0=ps.rearrange("p (y x) -> p y x", x=32),
                        in1=three, s0=bf13_sb[:, m:m + 1],
                        s1=1.0 / SQ["f1"], imm2=6.0,
                    )

                def dw_taps(dps_seg, h1w, m, base, u0, u1):
                    for p, (ta, tb) in enumerate(DW_PAIRS):
                        soa = tap_off(ta) + base
                        nc.tensor.matmul(
                            dps_seg[:, u0:u1],
                            wdgp_sb[:, m, 2 * p:2 * p + 2, :],
                            pair_ap(h1w[:, m, :], tap_off(tb) - tap_off(ta),
                                    soa + u0, soa + u1),
                            start=(p == 0), stop=False,
                            perf_mode=DR, skip_group_check=True,
                        )
                    for si, t in enumerate(DW_SINGLES):
                        so = tap_off(t) + base
                        nc.tensor.matmul(
                            dps_seg[:, u0:u1], wdgs_sb[:, m, si, :],
                            h1w[:, m, so + u0:so + u1],
                            start=False, stop=(si == 4),
                            skip_group_check=True,
                        )

                def hswish2(h2, m, half2, dps, nrows):
                    # half2: 0 or 1 (which 16-row half of the image)
                    nc.vector._custom_dve(
                        HSWISH6Q,
                        out=h2[:, m, half2 * 512:half2 * 512 + nrows * 32]
                        .rearrange("p (y x) -> p y x", x=32),
                        in0=dps.rearrange(
                            "p (y x) -> p y x", x=WP)[:, 0:nrows, 0:32],
                        in1=three, s0=bdw3_sb[:, m:m + 1],
                        s1=1.0 / SQ["dw"], imm2=6.0,
                    )

                def fc1dw_emit(b, m, dwpool):
                    # filler path: fc1 halves via misc, dw via two 2-bank psums
                    fc1_emit(b, m, 0, None, "misc")
                    fc1_emit(b, m, 1, None, "misc")
                    h1w = h1w_bufs[b]
                    if m == 0:
                        h2_tiles[b] = h2_pool.tile(
                            [128, 12, 1024], fp8, tag="h2", name="h2f")
                    h2 = h2_tiles[b]
                    for half2 in range(2):
                        dps = dwpool.tile([128, 640], f32, tag="dwp")
                        base = half2 * 16 * WP
                        for u0, u1 in SEGS2A:
                            dw_taps(dps, h1w, m, base, u0, u1)
                        hswish2(h2, m, half2, dps, 16)

                def fc2_emit(b, tl, pool, tag):
                    tg = b * 8 + tl
                    h2 = h2_tiles[b]
                    if tag == "misc":
                        psb = misc_tile()
                        ps = psb[:, 0:C]
                    else:
                        ps = pool.tile([128, C], f32, tag=tag)
                    for mi in range(6):
                        nc.tensor.matmul(
                            ps, h2[:, 2 * mi:2 * mi + 2,
                                   tl * 128:(tl + 1) * 128],
                            wf2_sb[:, 2 * mi:2 * mi + 2, :],
                            start=(mi == 0),
                            stop=(mi == 5 and not HAS_BIAS),
                            perf_mode=DR,
                        )
                    if HAS_BIAS:
                        nc.tensor.matmul(ps, ones8, bf2_sb,
                                         start=False, stop=True)
                    ot = out_pool.tile([128, C], f32, tag="out")
                    nc.vector.scalar_tensor_tensor(
                        ot, ps, 1.0 / SQ["f2"], x_sb[:, tg, :],
                        OP.mult, OP.add,
                    )
                    nc.sync.dma_start(
                        out=d["out"][b, tl * 128:(tl + 1) * 128, :],
                        in_=ot,
                    )

                def ln2_emit(b, g2=None):
                    gs = [g2] if g2 is not None else [0, 1]
                    for g in gs:
                        btts = list(range(b * 8 + g * 4, b * 8 + g * 4 + 4))
                        mvs, rstds = ln_stats(btts, ln2_pool)
                        for i, tt in enumerate(btts):
                            ln_tile(xn2_ch, tt, i, mvs, rstds, ln2_pool,
                                    misc_ps, xn_drain="dve")

                # ---- batch 0 attention: dense, double-buffered scores ----
                d0 = []
                with tc.tile_pool(name="st2_ps", bufs=2,
                                  space="PSUM") as st2_ps, \
                     tc.tile_pool(name="o0_ps", bufs=1,
                                  space="PSUM") as o0_ps:
                    attn_loop(0, [], st2_ps, o0_ps, defer=d0)

                # ---- batch 1 attention; batch 0 proj/LN2/IRB as filler ----
                with tc.tile_pool(name="st1_ps", bufs=1,
                                  space="PSUM") as st1_ps, \
                     tc.tile_pool(name="o1_ps", bufs=1,
                                  space="PSUM") as o1_ps, \
                     tc.tile_pool(name="dwf_ps", bufs=1,
                                  space="PSUM") as dwf_ps:
                    flr = [lambda h=h, ou=ou: norm_emit(0, h, ou)
                           for h, ou in d0]
                    flr += [lambda tl=tl: och_proj_emit(0, tl)
                            for tl in range(8)]
                    flr.append(lambda: ln2_emit(0))
                    for m in range(12):
                        flr.append(lambda m=m: fc1dw_emit(0, m, dwf_ps))
                    for tl in range(8):
                        flr.append(lambda tl=tl: fc2_emit(0, tl, None,
                                                          "misc"))
                    attn_loop(1, flr, st1_ps, o1_ps)
                    while flr:
                        flr.pop(0)()

                # batch-1 proj/LN2/IRB with the attention pools closed:
                # dedicated proj psum ring + double-buffered dw psum
                with tc.tile_pool(name="dwt_ps", bufs=2,
                                  space="PSUM") as dwt_ps, \
                     tc.tile_pool(name="pj2_ps", bufs=2,
                                  space="PSUM") as pj2_ps:
                    for tl in range(8):
                        och_proj_emit(1, tl, pjpool=pj2_ps)
                        if tl == 3:
                            ln2_emit(1, 0)
                    ln2_emit(1, 1)
                    for m in range(12):
                        fc1dw_emit(1, m, dwt_ps)
                    for tl in range(8):
                        fc2_emit(1, tl, None, "misc")


def declare_tensors(nc, scales, has_bias):
    d = {"scales": scales, "has_bias": has_bias}
    d["x"] = nc.dram_tensor("x", [BPC, N, C], bf16, kind="ExternalInput").ap()
    d["wqk"] = nc.dram_tensor("wqk", [4, 128, 2 * C], fp8, kind="ExternalInput").ap()
    d["bqk"] = nc.dram_tensor("bqk", [96, 8], f32, kind="ExternalInput").ap()
    d["wv"] = nc.dram_tensor("wv", [4, 128, C], fp8, kind="ExternalInput").ap()
    d["bv"] = nc.dram_tensor("bv", [1, C], fp8, kind="ExternalInput").ap()
    d["wp"] = nc.dram_tensor("wp", [4, 128, C], fp8, kind="ExternalInput").ap()
    d["bp"] = nc.dram_tensor("bp", [1, C], fp8, kind="ExternalInput").ap()
    d["wf1"] = nc.dram_tensor("wf1", [4, 128, HID], fp8, kind="ExternalInput").ap()
    d["bf13"] = nc.dram_tensor("bf13", [128, 12], f32, kind="ExternalInput").ap()
    d["wdgp"] = nc.dram_tensor("wdgp", [12, 4, 128, 128], fp8,
                               kind="ExternalInput").ap()
    d["wdgs"] = nc.dram_tensor("wdgs", [12, 5, 128, 128], fp8,
                               kind="ExternalInput").ap()
    d["bdw3"] = nc.dram_tensor("bdw3", [128, 12], f32, kind="ExternalInput").ap()
    d["wf2"] = nc.dram_tensor("wf2", [12, 128, C], fp8, kind="ExternalInput").ap()
    d["bf2"] = nc.dram_tensor("bf2", [1, C], fp8, kind="ExternalInput").ap()
    d["out"] = nc.dram_tensor("out", [BPC, N, C], f32, kind="ExternalOutput").ap()
    return d


@functools.lru_cache(maxsize=1)
def build_program(scale_items, has_bias=False, num_devices=NCORES):
    scales = dict(scale_items)
    nc = bacc.Bacc("TRN2", target_bir_lowering=False, debug=False,
                   num_devices=num_devices)
    d = declare_tensors(nc, scales, has_bias)
    with tile.TileContext(nc) as tc:
        emit_kernel(nc, tc, d)
    nc.compile()
    return nc


def _scale_for(w):
    m = float(np.abs(w).max())
    return 224.0 / m if m > 0 else 1.0


def prep_weights(inputs):
    """Host-side packing: transposes, LN folds, fp8 quantization + scales."""
    g1 = np.asarray(inputs["ln1_g"], np.float32)
    b1 = np.asarray(inputs["ln1_b"], np.float32)
    g2 = np.asarray(inputs["ln2_g"], np.float32)
    b2 = np.asarray(inputs["ln2_b"], np.float32)
    Wqkv = np.asarray(inputs["Wqkv"], np.float32)
    Wproj = np.asarray(inputs["Wproj"], np.float32)
    bproj = np.asarray(inputs["bproj"], np.float32)
    Wfc1 = np.asarray(inputs["Wfc1"], np.float32)[:, :, 0, 0]
    bfc1 = np.asarray(inputs["bfc1"], np.float32)
    Wdw = np.asarray(inputs["Wdw"], np.float32)[:, 0].reshape(HID, 9)
    bdw = np.asarray(inputs["bdw"], np.float32)
    Wfc2 = np.asarray(inputs["Wfc2"], np.float32)[:, :, 0, 0]
    bfc2 = np.asarray(inputs["bfc2"], np.float32)

    W3 = Wqkv.reshape(HEADS, 3, HD, C)      # out channel o = h*288 + s*96 + d
    scale = float(HD) ** -0.5
    Wq = W3[:, 0].reshape(HEADS * HD, C)
    Wk = W3[:, 1].reshape(HEADS * HD, C)
    Wv = W3[:, 2].reshape(HEADS * HD, C)

    wqk_full = np.concatenate([Wq * g1[None, :] * scale, Wk * g1[None, :]], 0)
    wv_full = Wv * g1[None, :]
    wf1_full = Wfc1 * g2[None, :]
    wdw_full = Wdw / 6.0
    wf2_full = Wfc2 / 6.0

    sc = {
        "qk": _scale_for(wqk_full), "v": _scale_for(wv_full),
        "p": _scale_for(Wproj), "f1": _scale_for(wf1_full),
        "dw": _scale_for(wdw_full), "f2": _scale_for(wf2_full),
    }

    d = {}
    def plane4(wt, ncols):
        w = np.zeros((4, 128, ncols), np.float32)
        w[0:3] = wt.T.reshape(3, 128, ncols)
        return w.astype(nf8)

    d["wqk"] = plane4(wqk_full * sc["qk"], 2 * C)
    d["bqk"] = np.ascontiguousarray(np.concatenate(
        [((Wq @ b1) * scale).reshape(HEADS, HD).T,
         (Wk @ b1).reshape(HEADS, HD).T], 1)).astype(np.float32)
    d["wv"] = plane4(wv_full * sc["v"], C)
    d["bv"] = ((Wv @ b1) * sc["v"])[None, :].astype(nf8)
    d["wp"] = plane4(Wproj * sc["p"], C)
    d["bp"] = (bproj * sc["p"])[None, :].astype(nf8)
    d["wf1"] = plane4(wf1_full * sc["f1"], HID)
    d["bf13"] = np.ascontiguousarray(
        (bfc1 + Wfc1 @ b2 + 3.0).reshape(12, 128).T).astype(np.float32)
    wd = wdw_full * sc["dw"]
    wdgp = np.zeros((12, 4, 128, 128), np.float32)
    wdgs = np.zeros((12, 5, 128, 128), np.float32)
    ii = np.arange(128)
    for m in range(12):
        for p, (ta, tb) in enumerate(DW_PAIRS):
            wdgp[m, 2 * p + 0, ii, ii] = wd[m * 128 + ii, ta]
            wdgp[m, 2 * p + 1, ii, ii] = wd[m * 128 + ii, tb]
        for si, t in enumerate(DW_SINGLES):
            wdgs[m, si, ii, ii] = wd[m * 128 + ii, t]
    d["wdgp"] = wdgp.astype(nf8)
    d["wdgs"] = wdgs.astype(nf8)
    d["bdw3"] = np.ascontiguousarray(
        (bdw + 3.0).reshape(12, 128).T).astype(np.float32)
    d["wf2"] = np.ascontiguousarray(
        (wf2_full * sc["f2"]).T.reshape(12, 128, C)).astype(nf8)
    d["bf2"] = (bfc2 * sc["f2"])[None, :].astype(nf8)
    has_bias = any(float(np.abs(v).max()) > 0 for v in
                   (Wv @ b1, bproj, bfc2))
    return d, sc, has_bias


def kernel(**inputs):
    from concourse.bass_utils import run_bass_kernel_spmd

    x = np.asarray(inputs["x"], np.float32)
    wd, sc, has_bias = prep_weights(inputs)
    nc = build_program(tuple(sorted(sc.items())), has_bias)
    in_maps = []
    for c in range(NCORES):
        m = dict(wd)
        m["x"] = np.ascontiguousarray(
            x[c * BPC:(c + 1) * BPC]).astype(nbf)
        in_maps.append(m)
    res = run_bass_kernel_spmd(nc, in_maps, list(range(NCORES)))
    out = np.concatenate([res.results[c]["out"] for c in range(NCORES)], axis=0)
    return out.astype(np.float32)


# revision 43
# speedup vs baseline: 1.0215x; 1.0025x over previous
"""Trainium2 Bass kernel for nn_Block_79680233275670 (dense transformer block).

Reference, for x [16, 1024, 384]:
  x = x + proj(attn(LN1(x)))                               (4 heads, head_dim 96)
  x = x + fc2(hswish(dw3x3(hswish(fc1(LN2(x))))))          (IRB, 32x32 spatial)

Sharding: pure data-parallel over batch B=16 -> 8 cores x 2 batch items.
No collectives. Weights replicated (pre-transposed / LN-folded / fp8-quantized
host-side).

Per-core dataflow (T = 2048 tokens = 2 batches x 1024):
  - x token-major [128, 16, 384] f32 (4 chunked DMAs); residual stream f32
  - LN token-major (bn_stats) -> bf16, PE-transpose -> ACT fp8 cast
  - fp8e4 DoubleRow matmuls (2 k-tiles/pass) for QKV / PV / proj / fc1 / fc2;
    scores q^T k stay bf16
  - per (batch, head): scores St[m,n] = k^T q in a double-buffered PSUM;
    exp on ACT emits fp8 P m-tile pairs = the DoubleRow rhs for PV. An
    appended ones column in v (padded to 112 rows) makes PV also emit
    softmax denominators. O-normalization interleaved per head: DVE drain,
    PE transposes into one [128,8,98] PSUM tile, one recip, per-slice mults
  - proj/fc2 token-major; fp8 dequant scale folded into the
    scalar_tensor_tensor residual add (x += psum*s)
  - IRB: fc1 channel-major fp8; hardswish = ONE custom DVE op
    (min(relu(x*s+b+3),6)*(relu(..)-3), /6 folded into dw / fc2 weights)
    writing fp8 windows directly; depthwise 3x3 on PE over 19-row
    zero-padded windows (40-elem row pitch): 2 DoubleRow tap pairs
    ((-1,dx),(+1,dx) for dx=+-1, 80-elem k-stride satisfies the 16B rule)
    + 5 single taps
"""

import sys
import functools

for _p in ("/opt/trn_rl_repo",):
    if _p not in sys.path:
        sys.path.insert(0, _p)

import numpy as np
import ml_dtypes

import concourse.bass as bass
import concourse.mybir as mybir
import concourse.tile as tile
from concourse import bacc
from concourse.ap import AP
from concourse.masks import make_identity


B, N, C = 16, 1024, 384
HEADS, HD = 4, 96
VP = 112                   # v rows incl. ones col, padded for DR alignment
HID = 1536
NCORES = 8
BPC = B // NCORES          # batches per core
T = BPC * N                # tokens per core
NT = T // 128              # 16 token tiles per core
EPS = 1e-5

f32 = mybir.dt.float32
bf16 = mybir.dt.bfloat16
fp8 = mybir.dt.float8e4
AF = mybir.ActivationFunctionType
OP = mybir.AluOpType
DR = mybir.MatmulPerfMode.DoubleRow
nbf = ml_dtypes.bfloat16
nf8 = ml_dtypes.float8_e4m3

# ---- custom fused hardswish DVE op (registered at import time) ------------
# out = min(relu(in*C1 + C0), 6) * (relu(in*C1 + C0) - 3)
#     = hswish6(in*C1 + (C0-3)) where hswish6(x) = x*clip(x+3,0,6)
# C0 = bias+3 (per-partition AP), C1 = fp8 dequant scale, C2 = 6 (imm2),
# C3 = 3 delivered via in1 (spilled).
import concourse.dve_ops as dve_ops
from concourse.dve_spec import Spec, Src0, C0, C1, C2, C3, relu, minn, lower
from concourse.dve_ops import DveOp, DveOpSpec, _spill_c3_to_src1


def _register_hswish():
    name = "HSWISH6Q_ANT"
    if name in dve_ops._SUB_OPCODE_FOR_NAME:
        for op in dve_ops.OPS:
            if op.name == name:
                return op
    r = relu(Src0 * C1 + C0)
    spec = Spec(
        body=_spill_c3_to_src1(minn(r, C2) * (r - C3)),
        reference=lambda in0, in1, s0, s1, imm2:
        np.minimum(np.maximum(in0 * s1 + s0, 0), imm2)
        * (np.maximum(in0 * s1 + s0, 0) - in1),
    )
    op = DveOp(name, spec, subdim=False, uops_sha={})
    row = dve_ops._CUSTOM_DVE_ROW_BASE + len(dve_ops.OPS)
    assert row < 0x20
    for ver in ("v3", "v4"):
        probe = DveOpSpec(name=name, opcode=row, uops=lower(spec, ver=ver),
                          rd1_en=True)
        op.uops_sha[ver] = probe.sha(ver)
    dve_ops.OPS.append(op)
    dve_ops._SUB_OPCODE_FOR_NAME[name] = row
    dve_ops.CUSTOM_DVE_SPECS[name] = spec
    return op


HSWISH6Q = _register_hswish()

# ---- depthwise-window geometry -------------------------------------------
# 19 rows per window: row 0 and row 18 stay zero (vertical SAME padding),
# rows 1..17 hold 17 image rows (16 outputs + 1 halo). Rows padded to WP=40
# (32 data + 8 zero cols -> horizontal SAME padding), plus HOFF=3 leading
# zeros. All 9 taps share identical geometry:
#   acc[0:AUSE) += w_t * win[so : so+AUSE),  so = HOFF+(dy+1+yh)*WP+dx
# Taps (-1,dx) and (+1,dx) pair into one DoubleRow matmul (k-stride 2*WP=80,
# even offsets for dx=+-1 since HOFF=3).
WP = 40
HOFF = 3
HLEN = HOFF + 34 * WP + 1  # rows: 1 zero + 32 image rows + 1 zero (pad->/4)
ACCL = 32 * WP             # 1280 acc length (full batch image)
AUSE = ACCL - 2            # 1278 initialized prefix
SEGS3 = ((0, 512), (512, 1024), (1024, AUSE))
SEGS2A = ((0, 512), (512, 638))          # half-image variants (2-bank psum)
DW_PAIRS = ((0, 6), (2, 8))      # (dy=-1,dx) + (dy=+1,dx) for dx=-1,+1
DW_SINGLES = (1, 3, 4, 5, 7)


def tap_off(t):
    # window row w holds image row w-1; acc row y reads window row y+dy+1
    dy, dx = t // 3 - 1, t % 3 - 1
    return HOFF + (dy + 1) * WP + dx


def pair_ap(w, delta, lo, hi):
    """[128, 2, hi-lo] view: two copies of w[:, lo:hi] offset by delta elems."""
    v = w[:, lo:hi]
    return AP(v.tensor, v.offset, [list(v.ap[0]), [delta, 2], [1, hi - lo]])


def emit_kernel(nc, tc, d):
    from contextlib import ExitStack

    with ExitStack() as ctx:
        singles = ctx.enter_context(tc.tile_pool(name="singles", bufs=1))

        x_sb = singles.tile([128, NT, C], bf16)  # token-major; becomes x2 in place
        ident = singles.tile([128, 128], bf16)
        make_identity(nc, ident)
        ones8 = singles.tile([1, 128], fp8)
        nc.vector.memset(ones8, 1.0)
        eps_sb = singles.tile([128, 1], f32)
        nc.vector.memset(eps_sb, EPS)
        three = singles.tile([128, 1], f32)
        nc.vector.memset(three, 3.0)

        wqk_sb = singles.tile([128, 4, 2 * C], fp8)
        bqk_sb = singles.tile([96, 8], f32)
        wv_sb = singles.tile([128, 4, C], fp8)
        bv_sb = singles.tile([1, C], fp8)
        wp_sb = singles.tile([128, 4, C], fp8)
        bp_sb = singles.tile([1, C], fp8)
        wf1_sb = singles.tile([128, 4, HID], fp8)
        bf13_sb = singles.tile([128, 12], f32)
        wdgp_sb = singles.tile([128, 12, 4, 128], fp8)
        wdgs_sb = singles.tile([128, 12, 5, 128], fp8)
        bdw3_sb = singles.tile([128, 12], f32)
        wf2_sb = singles.tile([128, 12, C], fp8)
        bf2_sb = singles.tile([1, C], fp8)

        xn2_ch = singles.tile([128, 4, T], fp8)
        nc.vector.memset(xn2_ch[:, 3, :], 0.0)
        h1w_a = singles.tile([128, 12, HLEN], fp8)
        h1w_b = singles.tile([128, 12, HLEN], fp8)
        h1w_bufs = [h1w_a, h1w_b]

        # x[b, i*128+p, c] -> x_sb[p, b*8+i, c], 8 chunks on 2 queues so
        # LN1 starts early and the load runs on two DMA paths
        for ch in range(8):
            eng = nc.sync if ch % 2 == 0 else nc.scalar
            eng.dma_start(
                out=x_sb[:, ch * 2:(ch + 1) * 2, :],
                in_=d["x"].rearrange("b (i p) c -> p (b i) c", p=128)
                [:, ch * 2:(ch + 1) * 2, :],
            )
        for name, dst in (("wqk", wqk_sb), ("wv", wv_sb), ("wp", wp_sb)):
            nc.sync.dma_start(out=dst, in_=d[name].rearrange("k p m -> p k m"))
        for name, dst in (("bqk", bqk_sb), ("bv", bv_sb), ("bp", bp_sb),
                          ("bf13", bf13_sb), ("bdw3", bdw3_sb),
                          ("bf2", bf2_sb)):
            nc.sync.dma_start(out=dst, in_=d[name])
        nc.sync.dma_start(out=wf1_sb, in_=d["wf1"].rearrange("k p m -> p k m"))
        nc.sync.dma_start(out=wf2_sb, in_=d["wf2"].rearrange("k p m -> p k m"))
        wdv_sb = singles.tile([128, 12, 9], f32)
        nc.scalar.dma_start(out=wdv_sb, in_=d["wdv"])

        SQ = d["scales"]  # dict of python floats
        HAS_BIAS = d["has_bias"]

        def ln_stats(tts, ln_pool):
            # one Sqrt + one reciprocal for the whole tile group
            G = len(tts)
            mvs = ln_pool.tile([128, 8, 2], f32, tag="ln_mvs")
            for i, tt in enumerate(tts):
                stats = ln_pool.tile([128, 6], f32, tag="ln_stats")
                nc.vector.bn_stats(stats, x_sb[:, tt, :])
                nc.vector.bn_aggr(mvs[:, i, :], stats)
            stds = ln_pool.tile([128, 8], f32, tag="ln_stds")
            nc.scalar.activation(stds[:, 0:G], mvs[:, 0:G, 1], AF.Sqrt,
                                 bias=eps_sb)
            rstds = ln_pool.tile([128, 8], f32, tag="ln_rstds")
            nc.vector.reciprocal(rstds[:, 0:G], stds[:, 0:G])
            return mvs, rstds

        def ln_tile(xn_ch, tt, i, mvs, rstds, ln_pool, ps_pool,
                    xn_drain=None, affine="dve"):
            xn = ln_pool.tile([128, C], bf16, tag="ln_xn")
            if affine == "act":
                negmr = ln_pool.tile([128, 1], f32, tag="ln_negmr")
                nc.vector.tensor_scalar(
                    negmr, mvs[:, i, 0:1], -1.0, rstds[:, i:i + 1],
                    OP.mult, OP.mult
                )
                nc.scalar.activation(xn, x_sb[:, tt, :], AF.Identity,
                                     bias=negmr, scale=rstds[:, i:i + 1])
            else:
                nc.vector.tensor_scalar(
                    xn, x_sb[:, tt, :], mvs[:, i, 0:1], rstds[:, i:i + 1],
                    OP.subtract, OP.mult
                )
            tpb = ps_pool.tile([128, 512], f32, tag="misc")
            tp = tpb.bitcast(bf16)[:, 0:C]
            for j in range(3):
                nc.tensor.transpose(
                    tp[:, j * 128:(j + 1) * 128],
                    xn[:, j * 128:(j + 1) * 128], ident,
                )
            dst = xn_ch[:, 0:3, tt * 128:(tt + 1) * 128]
            src = tp.rearrange("p (j t) -> p j t", j=3)
            if xn_drain == "dve":
                nc.vector.tensor_copy(dst, src)
            else:
                nc.scalar.activation(dst, src, AF.Copy)

        # ============ attention + IRB (software-pipelined) ============
        with tc.tile_pool(name="attn_acts", bufs=1) as apool, \
             tc.tile_pool(name="h2_pool", bufs=2) as h2_pool, \
             tc.tile_pool(name="out_pool", bufs=4) as out_pool:
            xn1_ch = apool.tile([128, 4, T], fp8)
            nc.vector.memset(xn1_ch[:, 3, :], 0.0)
            q_sb = apool.tile([96, HEADS, T], bf16)
            k_sb = apool.tile([96, HEADS, T], bf16)
            v_sb = apool.tile([128, NT, HEADS, VP], fp8)
            o_norm = apool.tile([128, NT, HEADS, HD], bf16)
            o_ch = apool.tile([128, BPC, 4, N], fp8)
            nc.vector.memset(o_ch[:, :, 3, :], 0.0)
            nc.vector.memset(v_sb[:, :, :, HD:HD + 1], 1.0)
            nc.vector.memset(v_sb[:, :, :, HD + 1:VP], 0.0)

            with tc.tile_pool(name="ln1", bufs=3) as ln_pool, \
                 tc.tile_pool(name="ln1_ps", bufs=3, space="PSUM") as lnps_pool:
                for g in range(4):
                    tts = list(range(g * 4, g * 4 + 4))
                    mvs, rstds = ln_stats(tts, ln_pool)
                    for i, tt in enumerate(tts):
                        ln_tile(xn1_ch, tt, i, mvs, rstds, ln_pool,
                                lnps_pool)

            nc.vector.memset(h1w_a.bitcast(f32), 0.0)
            nc.vector.memset(h1w_b.bitcast(f32), 0.0)


            def qk_emit(pool, tag, io, h, tk, width):
                dst = q_sb if io == 0 else k_sb
                co = io * C + h * HD
                if pool is None:
                    ps = misc_tile()[0:96, 0:width]
                else:
                    ps = pool.tile([96, width], f32, tag=tag)
                for half in range(width // 512):
                    tsl = slice(tk * width + half * 512,
                                tk * width + half * 512 + 512)
                    psl = slice(half * 512, half * 512 + 512)
                    nc.tensor.matmul(
                        ps[:, psl], wqk_sb[:, 0:2, co:co + HD],
                        xn1_ch[:, 0:2, tsl],
                        start=True, stop=False, perf_mode=DR,
                    )
                    nc.tensor.matmul(
                        ps[:, psl], wqk_sb[:, 2:4, co:co + HD],
                        xn1_ch[:, 2:4, tsl],
                        start=False, stop=True, perf_mode=DR,
                    )
                nc.scalar.activation(
                    dst[:, h, tk * width:(tk + 1) * width], ps, AF.Identity,
                    bias=bqk_sb[:, io * 4 + h: io * 4 + h + 1],
                    scale=1.0 / SQ["qk"],
                )

            def v_emit(pool, tag, tt):
                if pool is None:
                    ps = misc_tile()[:, 0:C]
                else:
                    ps = pool.tile([128, C], f32, tag=tag)
                tsl = slice(tt * 128, (tt + 1) * 128)
                nc.tensor.matmul(
                    ps, xn1_ch[:, 0:2, tsl], wv_sb[:, 0:2, :],
                    start=True, stop=False, perf_mode=DR,
                )
                nc.tensor.matmul(ps, xn1_ch[:, 2:4, tsl], wv_sb[:, 2:4, :],
                                 start=False, stop=not HAS_BIAS,
                                 perf_mode=DR)
                if HAS_BIAS:
                    nc.tensor.matmul(ps, ones8, bv_sb, start=False, stop=True)
                nc.vector.tensor_scalar(
                    v_sb[:, tt, :, 0:HD],
                    ps.rearrange("p (h e) -> p h e", h=HEADS),
                    1.0 / SQ["v"], None, OP.mult,
                )


            with tc.tile_pool(name="qk_ps", bufs=2, space="PSUM") as qk_ps, \
                 tc.tile_pool(name="v_ps", bufs=2, space="PSUM") as v_ps:
                for io in range(2):
                    for h in range(HEADS):
                        for cn in range(2):
                            qk_emit(qk_ps, "qk", io, h, cn, 1024)
                for tt in range(NT):
                    v_emit(v_ps, "v", tt)

            # build the depthwise diagonal weights on-chip (saves 1.8MB of
            # startup DMA): diag(w_t) = identity * per-channel scalar
            ident8 = singles.tile([128, 128], fp8)
            nc.scalar.activation(ident8, ident, AF.Copy)
            for m in range(12):
                for p, (ta, tb) in enumerate(DW_PAIRS):
                    for j, t in enumerate((ta, tb)):
                        nc.vector.tensor_scalar(
                            wdgp_sb[:, m, 2 * p + j, :], ident8,
                            wdv_sb[:, m, t:t + 1], None, OP.mult)
                for si, t in enumerate(DW_SINGLES):
                    nc.vector.tensor_scalar(
                        wdgs_sb[:, m, si, :], ident8,
                        wdv_sb[:, m, t:t + 1], None, OP.mult)

            with tc.tile_pool(name="misc_ps", bufs=2, space="PSUM") as misc_ps, \
                 tc.tile_pool(name="pt_pool", bufs=2) as pt_pool, \
                 tc.tile_pool(name="ou_pool", bufs=6) as ou_pool, \
                 tc.tile_pool(name="r_pool", bufs=4) as r_pool, \
                 tc.tile_pool(name="ln2", bufs=3) as ln2_pool:

                def misc_tile():
                    mt_ = misc_ps.tile([128, 512], f32, tag="misc")
                    return mt_

                def norm_emit(b, h, o_un):
                    tp8b = misc_tile()
                    tp8 = tp8b.bitcast(bf16)[:, 0:8 * (HD + 2)] \
                        .rearrange("p (a e) -> p a e", a=8)
                    for ns in range(8):
                        nc.tensor.transpose(
                            tp8[:, ns, 0:HD + 1],
                            o_un[0:HD + 1, ns * 128:(ns + 1) * 128],
                            ident[0:HD + 1, 0:HD + 1],
                        )
                    r8 = r_pool.tile([128, 8], f32, tag="r")
                    nc.vector.reciprocal(r8, tp8[:, :, HD])
                    for ns in range(8):
                        nc.vector.tensor_scalar(
                            o_norm[:, b * 8 + ns, h, :], tp8[:, ns, 0:HD],
                            r8[:, ns:ns + 1], None, OP.mult,
                        )

                def attn_loop(b, filler, st_ps, o_ps, defer=None):
                    def pop(k):
                        for _ in range(k):
                            if filler:
                                filler.pop(0)()
                    for h in range(HEADS):
                        o_psum = o_ps.tile([VP, N], f32, tag="o")
                        for mp in range(4):
                            pt2 = pt_pool.tile([128, 2, N], fp8, tag="pt")
                            for i in range(2):
                                mt = 2 * mp + i
                                st = st_ps.tile([128, N], f32, tag="st")
                                for cn in range(2):
                                    nc.tensor.matmul(
                                        st[:, cn * 512:(cn + 1) * 512],
                                        k_sb[:, h, b * N + mt * 128:
                                             b * N + (mt + 1) * 128],
                                        q_sb[:, h, b * N + cn * 512:
                                             b * N + (cn + 1) * 512],
                                        start=True, stop=True,
                                    )
                                nc.scalar.activation(pt2[:, i, :], st, AF.Exp)
                                pop(1)
                            vp = v_sb[:, b * 8 + 2 * mp: b * 8 + 2 * mp + 2,
                                      h, :]
                            for cn in range(2):
                                nc.tensor.matmul(
                                    o_psum[:, cn * 512:(cn + 1) * 512],
                                    vp, pt2[:, :, cn * 512:(cn + 1) * 512],
                                    start=(mp == 0), stop=(mp == 3),
                                    perf_mode=DR, skip_group_check=True,
                                )
                            pop(1)
                        o_un = ou_pool.tile([VP, N], bf16, tag="ou")
                        nc.vector.tensor_copy(o_un, o_psum)
                        if defer is not None:
                            defer.append((h, o_un))
                        else:
                            norm_emit(b, h, o_un)
                        pop(2)

                def och_proj_emit(b, tl, pjpool=None):
                    tt = b * 8 + tl
                    otb = misc_tile()
                    ot = otb.bitcast(bf16)[:, 0:C]
                    ov = o_norm[:, tt, :, :].rearrange("p h e -> p (h e)")
                    for j in range(3):
                        nc.tensor.transpose(
                            ot[:, j * 128:(j + 1) * 128],
                            ov[:, j * 128:(j + 1) * 128], ident,
                        )
                    nc.scalar.activation(
                        o_ch[:, b, 0:3, tl * 128:(tl + 1) * 128],
                        ot.rearrange("p (j t) -> p j t", j=3),
                        AF.Copy,
                    )
                    if pjpool is not None:
                        psb = pjpool.tile([128, 512], f32, tag="pj2")
                    else:
                        psb = misc_tile()
                    ps = psb[:, 0:C]
                    tsl = slice(tl * 128, (tl + 1) * 128)
                    nc.tensor.matmul(
                        ps, o_ch[:, b, 0:2, tsl], wp_sb[:, 0:2, :],
                        start=True, stop=False, perf_mode=DR,
                    )
                    nc.tensor.matmul(ps, o_ch[:, b, 2:4, tsl],
                                     wp_sb[:, 2:4, :],
                                     start=False, stop=not HAS_BIAS,
                                     perf_mode=DR)
                    if HAS_BIAS:
                        nc.tensor.matmul(ps, ones8, bp_sb,
                                         start=False, stop=True)
                    nc.vector.scalar_tensor_tensor(
                        x_sb[:, tt, :], ps, 1.0 / SQ["p"], x_sb[:, tt, :],
                        OP.mult, OP.add,
                    )

                h2_tiles = {}

                def fc1_emit(b, m, half, pool, tag):
                    # fc1 for 512 tokens (16 image rows) -> hswish -> window
                    h1w = h1w_bufs[b]
                    msl = slice(m * 128, (m + 1) * 128)
                    c0 = half * 512
                    if tag == "misc":
                        psb = misc_tile()
                        ps = psb[:, 0:512]
                    else:
                        ps = pool.tile([128, 512], f32, tag=tag)
                    csl = slice(b * N + c0, b * N + c0 + 512)
                    nc.tensor.matmul(
                        ps, wf1_sb[:, 0:2, msl], xn2_ch[:, 0:2, csl],
                        start=True, stop=False, perf_mode=DR,
                    )
                    nc.tensor.matmul(
                        ps, wf1_sb[:, 2:4, msl], xn2_ch[:, 2:4, csl],
                        start=False, stop=True, perf_mode=DR,
                    )
                    lo = HOFF + (1 + 16 * half) * WP
                    h1v = h1w[:, m, lo:lo + 16 * WP] \
                        .rearrange("p (y x) -> p y x", x=WP)[:, :, 0:32]
                    nc.vector._custom_dve(
                        HSWISH6Q, out=h1v,
                        in0=ps.rearrange("p (y x) -> p y x", x=32),
                        in1=three, s0=bf13_sb[:, m:m + 1],
                        s1=1.0 / SQ["f1"], imm2=6.0,
                    )

                def dw_taps(dps_seg, h1w, m, base, u0, u1):
                    for p, (ta, tb) in enumerate(DW_PAIRS):
                        soa = tap_off(ta) + base
                        nc.tensor.matmul(
                            dps_seg[:, u0:u1],
                            wdgp_sb[:, m, 2 * p:2 * p + 2, :],
                            pair_ap(h1w[:, m, :], tap_off(tb) - tap_off(ta),
                                    soa + u0, soa + u1),
                            start=(p == 0), stop=False,
                            perf_mode=DR, skip_group_check=True,
                        )
                    for si, t in enumerate(DW_SINGLES):
                        so = tap_off(t) + base
                        nc.tensor.matmul(
                            dps_seg[:, u0:u1], wdgs_sb[:, m, si, :],
                            h1w[:, m, so + u0:so + u1],
                            start=False, stop=(si == 4),
                            skip_group_check=True,
                        )

                def hswish2(h2, m, half2, dps, nrows):
                    # half2: 0 or 1 (which 16-row half of the image)
                    nc.vector._custom_dve(
                        HSWISH6Q,
                        out=h2[:, m, half2 * 512:half2 * 512 + nrows * 32]
                        .rearrange("p (y x) -> p y x", x=32),
                        in0=dps.rearrange(
                            "p (y x) -> p y x", x=WP)[:, 0:nrows, 0:32],
                        in1=three, s0=bdw3_sb[:, m:m + 1],
                        s1=1.0 / SQ["dw"], imm2=6.0,
                    )

                def fc1dw_emit(b, m, dwpool):
                    # filler path: fc1 halves via misc, dw via two 2-bank psums
                    fc1_emit(b, m, 0, None, "misc")
                    fc1_emit(b, m, 1, None, "misc")
                    h1w = h1w_bufs[b]
                    if m == 0:
                        h2_tiles[b] = h2_pool.tile(
                            [128, 12, 1024], fp8, tag="h2", name="h2f")
                    h2 = h2_tiles[b]
                    for half2 in range(2):
                        dps = dwpool.tile([128, 640], f32, tag="dwp")
                        base = half2 * 16 * WP
                        for u0, u1 in SEGS2A:
                            dw_taps(dps, h1w, m, base, u0, u1)
                        hswish2(h2, m, half2, dps, 16)

                def fc2_emit(b, tl, pool, tag):
                    tg = b * 8 + tl
                    h2 = h2_tiles[b]
                    if tag == "misc":
                        psb = misc_tile()
                        ps = psb[:, 0:C]
                    else:
                        ps = pool.tile([128, C], f32, tag=tag)
                    for mi in range(6):
                        nc.tensor.matmul(
                            ps, h2[:, 2 * mi:2 * mi + 2,
                                   tl * 128:(tl + 1) * 128],
                            wf2_sb[:, 2 * mi:2 * mi + 2, :],
                            start=(mi == 0),
                            stop=(mi == 5 and not HAS_BIAS),
                            perf_mode=DR,
                        )
                    if HAS_BIAS:
                        nc.tensor.matmul(ps, ones8, bf2_sb,
                                         start=False, stop=True)
                    ot = out_pool.tile([128, C], f32, tag="out")
                    nc.vector.scalar_tensor_tensor(
                        ot, ps, 1.0 / SQ["f2"], x_sb[:, tg, :],
                        OP.mult, OP.add,
                    )
                    nc.sync.dma_start(
                        out=d["out"][b, tl * 128:(tl + 1) * 128, :],
                        in_=ot,
                    )

                def ln2_emit(b, g2=None):
                    gs = [g2] if g2 is not None else [0, 1]
                    for g in gs:
                        btts = list(range(b * 8 + g * 4, b * 8 + g * 4 + 4))
                        mvs, rstds = ln_stats(btts, ln2_pool)
                        for i, tt in enumerate(btts):
                            ln_tile(xn2_ch, tt, i, mvs, rstds, ln2_pool,
                                    misc_ps, xn_drain="dve")

                # ---- batch 0 attention: dense, double-buffered scores ----
                d0 = []
                with tc.tile_pool(name="st2_ps", bufs=2,
                                  space="PSUM") as st2_ps, \
                     tc.tile_pool(name="o0_ps", bufs=1,
                                  space="PSUM") as o0_ps:
                    attn_loop(0, [], st2_ps, o0_ps, defer=d0)

                # ---- batch 1 attention; batch 0 proj/LN2/IRB as filler ----
                with tc.tile_pool(name="st1_ps", bufs=1,
                                  space="PSUM") as st1_ps, \
                     tc.tile_pool(name="o1_ps", bufs=1,
                                  space="PSUM") as o1_ps, \
                     tc.tile_pool(name="dwf_ps", bufs=1,
                                  space="PSUM") as dwf_ps:
                    flr = [lambda h=h, ou=ou: norm_emit(0, h, ou)
                           for h, ou in d0]
                    flr += [lambda tl=tl: och_proj_emit(0, tl)
                            for tl in range(8)]
                    flr.append(lambda: ln2_emit(0))
                    for m in range(12):
                        flr.append(lambda m=m: fc1dw_emit(0, m, dwf_ps))
                    for tl in range(8):
                        flr.append(lambda tl=tl: fc2_emit(0, tl, None,
                                                          "misc"))
                    attn_loop(1, flr, st1_ps, o1_ps)
                    while flr:
                        flr.pop(0)()

                # batch-1 proj/LN2/IRB with the attention pools closed:
                # dedicated proj psum ring + double-buffered dw psum
                with tc.tile_pool(name="dwt_ps", bufs=2,
                                  space="PSUM") as dwt_ps, \
                     tc.tile_pool(name="pj2_ps", bufs=2,
                                  space="PSUM") as pj2_ps:
                    for tl in range(8):
                        och_proj_emit(1, tl, pjpool=pj2_ps)
                        if tl == 3:
                            ln2_emit(1, 0)
                    ln2_emit(1, 1)
                    for m in range(12):
                        fc1dw_emit(1, m, dwt_ps)
                    for tl in range(8):
                        fc2_emit(1, tl, None, "misc")


def declare_tensors(nc, scales, has_bias):
    d = {"scales": scales, "has_bias": has_bias}
    d["x"] = nc.dram_tensor("x", [BPC, N, C], bf16, kind="ExternalInput").ap()
    d["wqk"] = nc.dram_tensor("wqk", [4, 128, 2 * C], fp8, kind="ExternalInput").ap()
    d["bqk"] = nc.dram_tensor("bqk", [96, 8], f32, kind="ExternalInput").ap()
    d["wv"] = nc.dram_tensor("wv", [4, 128, C], fp8, kind="ExternalInput").ap()
    d["bv"] = nc.dram_tensor("bv", [1, C], fp8, kind="ExternalInput").ap()
    d["wp"] = nc.dram_tensor("wp", [4, 128, C], fp8, kind="ExternalInput").ap()
    d["bp"] = nc.dram_tensor("bp", [1, C], fp8, kind="ExternalInput").ap()
    d["wf1"] = nc.dram_tensor("wf1", [4, 128, HID], fp8, kind="ExternalInput").ap()
    d["bf13"] = nc.dram_tensor("bf13", [128, 12], f32, kind="ExternalInput").ap()
    d["wdv"] = nc.dram_tensor("wdv", [128, 12, 9], f32,
                              kind="ExternalInput").ap()
    d["bdw3"] = nc.dram_tensor("bdw3", [128, 12], f32, kind="ExternalInput").ap()
    d["wf2"] = nc.dram_tensor("wf2", [12, 128, C], fp8, kind="ExternalInput").ap()
    d["bf2"] = nc.dram_tensor("bf2", [1, C], fp8, kind="ExternalInput").ap()
    d["out"] = nc.dram_tensor("out", [BPC, N, C], f32, kind="ExternalOutput").ap()
    return d


@functools.lru_cache(maxsize=1)
def build_program(scale_items, has_bias=False, num_devices=NCORES):
    scales = dict(scale_items)
    nc = bacc.Bacc("TRN2", target_bir_lowering=False, debug=False,
                   num_devices=num_devices)
    d = declare_tensors(nc, scales, has_bias)
    with tile.TileContext(nc) as tc:
        emit_kernel(nc, tc, d)
    nc.compile()
    return nc


def _scale_for(w):
    m = float(np.abs(w).max())
    return 224.0 / m if m > 0 else 1.0


def prep_weights(inputs):
    """Host-side packing: transposes, LN folds, fp8 quantization + scales."""
    g1 = np.asarray(inputs["ln1_g"], np.float32)
    b1 = np.asarray(inputs["ln1_b"], np.float32)
    g2 = np.asarray(inputs["ln2_g"], np.float32)
    b2 = np.asarray(inputs["ln2_b"], np.float32)
    Wqkv = np.asarray(inputs["Wqkv"], np.float32)
    Wproj = np.asarray(inputs["Wproj"], np.float32)
    bproj = np.asarray(inputs["bproj"], np.float32)
    Wfc1 = np.asarray(inputs["Wfc1"], np.float32)[:, :, 0, 0]
    bfc1 = np.asarray(inputs["bfc1"], np.float32)
    Wdw = np.asarray(inputs["Wdw"], np.float32)[:, 0].reshape(HID, 9)
    bdw = np.asarray(inputs["bdw"], np.float32)
    Wfc2 = np.asarray(inputs["Wfc2"], np.float32)[:, :, 0, 0]
    bfc2 = np.asarray(inputs["bfc2"], np.float32)

    W3 = Wqkv.reshape(HEADS, 3, HD, C)      # out channel o = h*288 + s*96 + d
    scale = float(HD) ** -0.5
    Wq = W3[:, 0].reshape(HEADS * HD, C)
    Wk = W3[:, 1].reshape(HEADS * HD, C)
    Wv = W3[:, 2].reshape(HEADS * HD, C)

    wqk_full = np.concatenate([Wq * g1[None, :] * scale, Wk * g1[None, :]], 0)
    wv_full = Wv * g1[None, :]
    wf1_full = Wfc1 * g2[None, :]
    wdw_full = Wdw / 6.0
    wf2_full = Wfc2 / 6.0

    sc = {
        "qk": _scale_for(wqk_full), "v": _scale_for(wv_full),
        "p": _scale_for(Wproj), "f1": _scale_for(wf1_full),
        "dw": _scale_for(wdw_full), "f2": _scale_for(wf2_full),
    }

    d = {}
    def plane4(wt, ncols):
        w = np.zeros((4, 128, ncols), np.float32)
        w[0:3] = wt.T.reshape(3, 128, ncols)
        return w.astype(nf8)

    d["wqk"] = plane4(wqk_full * sc["qk"], 2 * C)
    d["bqk"] = np.ascontiguousarray(np.concatenate(
        [((Wq @ b1) * scale).reshape(HEADS, HD).T,
         (Wk @ b1).reshape(HEADS, HD).T], 1)).astype(np.float32)
    d["wv"] = plane4(wv_full * sc["v"], C)
    d["bv"] = ((Wv @ b1) * sc["v"])[None, :].astype(nf8)
    d["wp"] = plane4(Wproj * sc["p"], C)
    d["bp"] = (bproj * sc["p"])[None, :].astype(nf8)
    d["wf1"] = plane4(wf1_full * sc["f1"], HID)
    d["bf13"] = np.ascontiguousarray(
        (bfc1 + Wfc1 @ b2 + 3.0).reshape(12, 128).T).astype(np.float32)
    wd = wdw_full * sc["dw"]
    d["wdv"] = np.ascontiguousarray(
        wd.reshape(12, 128, 9).transpose(1, 0, 2)).astype(np.float32)
    d["bdw3"] = np.ascontiguousarray(
        (bdw + 3.0).reshape(12, 128).T).astype(np.float32)
    d["wf2"] = np.ascontiguousarray(
        (wf2_full * sc["f2"]).T.reshape(12, 128, C)).astype(nf8)
    d["bf2"] = (bfc2 * sc["f2"])[None, :].astype(nf8)
    has_bias = any(float(np.abs(v).max()) > 0 for v in
                   (Wv @ b1, bproj, bfc2))
    return d, sc, has_bias


def kernel(**inputs):
    from concourse.bass_utils import run_bass_kernel_spmd

    x = np.asarray(inputs["x"], np.float32)
    wd, sc, has_bias = prep_weights(inputs)
    nc = build_program(tuple(sorted(sc.items())), has_bias)
    in_maps = []
    for c in range(NCORES):
        m = dict(wd)
        m["x"] = np.ascontiguousarray(
            x[c * BPC:(c + 1) * BPC]).astype(nbf)
        in_maps.append(m)
    res = run_bass_kernel_spmd(nc, in_maps, list(range(NCORES)))
    out = np.concatenate([res.results[c]["out"] for c in range(NCORES)], axis=0)
    return out.astype(np.float32)
